# revision 18
# baseline (speedup 1.0000x reference)
"""Fused CE + supervised-contrastive loss on 8 Trainium2 NeuronCores.

Math (reference semantics):
  ce   = -mean_i log_softmax(input)[i, y_i]
  sim  = (X @ X.T) / tau, diag excluded
  lse_i = logsumexp_{k!=i} sim[i,k]
  possum_i = (x_i . S_{y_i} - ||x_i||^2)/tau, S_c = sum_{k: y_k=c} x_k
  per_i = lse_i - possum_i/n_pos_i  (0 if n_pos_i == 0)
  loss = (1-lmbd)*ce + lmbd * sum_i per_i

Distribution: each core owns 1024 rows, streams all 8192 columns. The only
O(N^2) work is the row-wise sum of exp(sim); it is split across TWO engines
working in parallel on a shared 3-slot PSUM rotation (6 banks):

  * ACT path (cols 0..ACOLS): row-major sim chunks [128 rows, 1024 cols].
    ACT exp with bias=-100 and fused accum_out row-sums (esum). The diagonal
    (always in local cols [0,1024) thanks to the per-core xt2 rotation) is
    killed pre-exp by a diag(-1e4) accumulate-matmul, as in the classic
    flash pattern.
  * DVE path (cols ACOLS..8192): TRANSPOSED sim chunks [128 cols, 1024 rows]
    (lhsT = xt2 column block, rhs = xbt). DVE computes a one-instruction
    Schraudolph fast-exp: bits16 = trunc(max(sim,0) * 128/ln2) as int16,
    bitcast to bf16 == e^(sim - 127*ln2) * rho, rho in [1, 1.086]
    (measured mean 1.0410 on HW, folded into KDV below). The PE then
    row-sums those bf16 tiles with ones-lhsT matmuls into a persistent
    PSUM accumulator (4 sub-accumulators at partitions {0,32,64,96} for
    the 2 K-halves x 2 row-halves), giving rowsums with rows in the FREE
    dim; a 4x PE-transpose at the tail converts to row-major [128, 8].
    The max(.,0) clamp is mandatory: the int16 convert WRAPS on negative
    (clamped terms contribute exactly +0.0, an error < e^-88 per term).

  PE lanes: ACT fills run on array rows 0-63 (lo operand copies), DVE fills
  on rows 64-127 (hi copies) -- concurrent via tile_position row groups.
  The ones-reduces split K=64 lo/hi the same way.

  Combine: se_i = esum_i + KDV * seD_i with KDV = e^(127*ln2 - 100)/1.0410
  (shift alignment + measured Schraudolph mean bias).

Class sums S (and class counts, S's last column) are computed per-core and
AllReduce'd first thing -- the collective's ncfw/barrier latency overlaps
the exp phase; everything S-dependent (G gather, possum, counts) sits at
the tail.

PSUM (8 banks): 3x [128,1024] f32 stream slots (6) + rowsum accum [97,512]
(1) + aux rotation S->G->cnt->transpose (1).
"""

import math

import numpy as np

N, C = 8192, 64
NCORES = 8
RPC = N // NCORES          # rows per core (1024)
P = 128                    # partitions per row-block
NBLK = RPC // P            # 8 row blocks per core
TAU = 0.5
LMBD = 0.5
SHIFT = 100.0              # ACT-path logsumexp shift
ACOLS = 4096               # ACT row-major columns per core
DCOLS = N - ACOLS          # DVE transposed columns per core
AW = 1024                  # ACT chunk width
NACH = ACOLS // AW         # ACT chunks per block (4)
NKB = DCOLS // P           # DVE column blocks (32)
AEXP = 128.0 / math.log(2.0)          # fast-exp scale (184.6646)
RHO = 1.0410                          # measured mean Schraudolph ratio (HW)
KDV = math.exp(127.0 * math.log(2.0) - SHIFT) / RHO

_CACHE = {}


def _build():
    from contextlib import ExitStack

    import concourse.bass as bass
    import concourse.tile as tile
    from concourse import bacc, mybir

    f32 = mybir.dt.float32
    i16 = mybir.dt.int16
    i32 = mybir.dt.int32
    bf16 = mybir.dt.bfloat16
    AF = mybir.ActivationFunctionType
    ALU = mybir.AluOpType
    AX = mybir.AxisListType

    nc = bacc.Bacc(
        "TRN2",
        target_bir_lowering=False,
        debug=False,
        num_devices=NCORES,
    )

    xt2a_d = nc.dram_tensor("xt2a", [C, ACOLS], bf16, kind="ExternalInput")
    xt2b_d = nc.dram_tensor("xt2b", [C, DCOLS], bf16, kind="ExternalInput")
    xbt_d = nc.dram_tensor("xbt", [C, RPC], bf16, kind="ExternalInput")
    xaug_d = nc.dram_tensor("xaug", [P, NBLK * (C + 1)], bf16, kind="ExternalInput")
    ohb_d = nc.dram_tensor("ohb", [P, NBLK * C], bf16, kind="ExternalInput")
    ohbt_d = nc.dram_tensor("ohbt", [C, RPC], bf16, kind="ExternalInput")
    eye_d = nc.dram_tensor("eyeneg", [P, P], bf16, kind="ExternalInput")
    idn_d = nc.dram_tensor("ident", [P, P], bf16, kind="ExternalInput")
    out_d = nc.dram_tensor("out", [P, 2], f32, kind="ExternalOutput")
    dbg_d = nc.dram_tensor("dbg", [P, 1600], f32, kind="ExternalOutput")

    def emit(tc, ctx):
        const = ctx.enter_context(tc.tile_pool(name="const", bufs=1))
        dram = ctx.enter_context(tc.tile_pool(name="dram", bufs=1, space="DRAM"))
        strm = ctx.enter_context(tc.tile_pool(name="strm", bufs=3, space="PSUM"))
        accp = ctx.enter_context(tc.tile_pool(name="accp", bufs=1, space="PSUM"))
        auxp = ctx.enter_context(tc.tile_pool(name="auxp", bufs=1, space="PSUM"))
        scrp = ctx.enter_context(tc.tile_pool(name="scrp", bufs=2))
        ep = ctx.enter_context(tc.tile_pool(name="ep", bufs=3))
        p3s = ctx.enter_context(tc.tile_pool(name="p3s", bufs=2))
        stats = ctx.enter_context(tc.tile_pool(name="stats", bufs=1))

        # ---- input DMAs: S-path operands lead the sync queue so every
        # core reaches the AllReduce within ~2us (the collective barrier
        # waits for the slowest core); big xt2 streams behind / parallel ----
        xaug_sb = const.tile([P, NBLK * (C + 1)], bf16)
        nc.sync.dma_start(xaug_sb[:], xaug_d.ap())
        ohb_sb = const.tile([P, NBLK * C], bf16)
        nc.sync.dma_start(ohb_sb[:], ohb_d.ap())

        xbt_sb = const.tile([P, RPC], bf16)
        nc.sync.dma_start(xbt_sb[0:C, :], xbt_d.ap())
        nc.sync.dma_start(xbt_sb[C:P, :], xbt_d.ap())
        xt2b_sb = const.tile([P, DCOLS], bf16)
        nc.gpsimd.dma_start(xt2b_sb[C:P, 0:AW], xt2b_d.ap()[:, 0:AW])
        xt2a_sb = const.tile([P, ACOLS], bf16)
        nc.sync.dma_start(xt2a_sb[0:C, 0:AW], xt2a_d.ap()[:, 0:AW])
        eye_sb = const.tile([P, P], bf16)
        nc.sync.dma_start(eye_sb[:], eye_d.ap())
        idn_sb = const.tile([P, P], bf16)
        nc.sync.dma_start(idn_sb[:], idn_d.ap())
        nc.sync.dma_start(xt2a_sb[0:C, AW:ACOLS], xt2a_d.ap()[:, AW:ACOLS])
        nc.gpsimd.dma_start(xt2b_sb[C:P, AW:DCOLS], xt2b_d.ap()[:, AW:DCOLS])
        ohbt_sb = const.tile([C, RPC], bf16)
        nc.gpsimd.dma_start(ohbt_sb[:], ohbt_d.ap())

        # ---- persistent small tiles ----
        nshift = stats.tile([P, 1], f32)
        nc.vector.memset(nshift[:], -SHIFT)
        # masked-ones lhsT for the rowsum reduce: mh0 selects out rows 0-63
        # (row-half 0), mh1 rows 64-127 (half 1); both sum all 128 k
        masks = stats.tile([P, 2 * P], bf16)
        nc.vector.memset(masks[:, 0:C], 1.0)
        nc.vector.memset(masks[:, C:P], 0.0)
        nc.vector.memset(masks[:, P : P + C], 0.0)
        nc.vector.memset(masks[:, P + C : 2 * P], 1.0)
        esum = stats.tile([P, NBLK * NACH], f32)
        acc_sb = stats.tile([P, 512], bf16)
        se = stats.tile([P, NBLK], f32)
        seD = stats.tile([P, NBLK], f32)
        nrm = stats.tile([P, NBLK], f32)
        poss = stats.tile([P, NBLK], f32)
        cnt = stats.tile([P, NBLK], f32)
        lgt = stats.tile([P, NBLK], f32)
        cesum = stats.tile([P, NBLK], f32)
        s_loc = stats.tile([C, C + 1], f32)
        s_sb = stats.tile([C, C + 1], bf16)
        res = stats.tile([P, 2], f32)

        # rowsum accumulator: rows 0-63 hold row-half 0 sums (redundant
        # copies), rows 64-127 half 1; free = row-within-half
        acc = accp.tile([P, 512], f32)

        # ---- class sums S + AllReduce, first compute on the device ----
        s_ps = auxp.tile([P, 512], f32, tag="aux")
        s_view = s_ps[:C, : C + 1]
        for b in range(NBLK):
            nc.tensor.matmul(
                s_view,
                lhsT=ohb_sb[:, b * C : (b + 1) * C],
                rhs=xaug_sb[:, b * (C + 1) : (b + 1) * (C + 1)],
                start=(b == 0),
                stop=(b == NBLK - 1),
            )
        nc.vector.tensor_copy(s_loc[:], s_view)
        s_in = dram.tile([C, C + 1], f32)
        s_out = dram.tile([C, C + 1], f32)
        nc.sync.dma_start(s_in[:], s_loc[:])
        nc.gpsimd.collective_compute(
            "AllReduce",
            mybir.AluOpType.add,
            replica_groups=[list(range(NCORES))],
            ins=[s_in.opt()],
            outs=[s_out.opt()],
        )
        nc.gpsimd.dma_start(s_sb[:], s_out[:])  # casts f32 -> bf16

        # ---- early DVE small-ops (fit into stream slack) ----
        xaug_blk = xaug_sb[:].rearrange("p (b c) -> p b c", c=C + 1)[:, :, 0:C]
        ohb_blk = ohb_sb[:].rearrange("p (b c) -> p b c", c=C)
        t0 = p3s.tile([P, NBLK * C], f32, tag="p3")
        t0_blk = t0[:].rearrange("p (b c) -> p b c", c=C)
        nc.vector.tensor_mul(t0_blk, xaug_blk, xaug_blk)
        nc.vector.reduce_sum(nrm[:], t0_blk, axis=AX.X)
        t1 = p3s.tile([P, NBLK * C], f32, tag="p3")
        t1_blk = t1[:].rearrange("p (b c) -> p b c", c=C)
        nc.vector.tensor_mul(t1_blk, xaug_blk, ohb_blk)
        nc.vector.reduce_sum(lgt[:], t1_blk, axis=AX.X)
        # CE denominators: one batched exp (incl. the ones column; skipped in
        # the reduce via a strided AP)
        cescr = p3s.tile([P, NBLK * (C + 1)], f32, tag="ce")
        nc.scalar.activation(cescr[:], xaug_sb[:], AF.Exp)
        nc.vector.reduce_sum(
            cesum[:],
            cescr[:].rearrange("p (b c) -> p b c", c=C + 1)[:, :, 0:C],
            axis=AX.X,
        )

        # ---- main interleaved exp stream ----
        # Each super-step processes one ACT chunk (row-major, lo operands,
        # PE row-group 0) and one DVE chunk (transposed, hi operands, PE
        # row-group 64) with fills interleaved so adjacent matmuls target
        # different row groups and run concurrently.  The masked-ones
        # rowsum-reduce matmuls of the PREVIOUS DVE chunk are emitted after
        # this step's fills so the in-order PE queue never waits on DVE.
        pending_acc = []

        def emit_acc(kb, eb):
            for h in range(2):
                nc.tensor.matmul(
                    acc[:, 0:512],
                    lhsT=masks[:, h * P : (h + 1) * P],
                    rhs=eb[:, h * 512 : (h + 1) * 512],
                    start=(kb == 0 and h == 0),
                    stop=(kb == NKB - 1 and h == 1),
                    skip_group_check=True,
                )

        def super_step(b, w, kb):
            ps_a = strm.tile([P, AW], f32, tag="s")
            ps_d = strm.tile([P, AW], f32, tag="s")
            for h in range(2):
                nc.tensor.matmul(
                    ps_a[:, h * 512 : (h + 1) * 512],
                    lhsT=xbt_sb[0:C, b * P : (b + 1) * P],
                    rhs=xt2a_sb[0:C, w * AW + h * 512 : w * AW + (h + 1) * 512],
                    start=True,
                    stop=True,
                )
                nc.tensor.matmul(
                    ps_d[:, h * 512 : (h + 1) * 512],
                    lhsT=xt2b_sb[C:P, kb * P : (kb + 1) * P],
                    rhs=xbt_sb[C:P, h * 512 : (h + 1) * 512],
                    start=True,
                    stop=True,
                )
            if w == 0:
                # kill self-similarity (local col b*128+p) pre-exp
                nc.tensor.matmul(
                    ps_a[:, b * P : (b + 1) * P],
                    lhsT=idn_sb[:],
                    rhs=eye_sb[:],
                    start=False,
                    stop=True,
                    skip_group_check=True,
                )
            if pending_acc:
                emit_acc(*pending_acc.pop())
            scr = scrp.tile([P, AW], bf16, tag="scr")
            idx = b * NACH + w
            nc.scalar.activation(
                scr[:], ps_a[:], AF.Exp, bias=nshift[:],
                accum_out=esum[:, idx : idx + 1],
            )
            et = ep.tile([P, AW], i16, tag="E")
            nc.vector.tensor_scalar(
                out=et[:], in0=ps_d[:], scalar1=0.0, scalar2=AEXP,
                op0=ALU.max, op1=ALU.mult,
            )
            pending_acc.append((kb, et[:].bitcast(bf16)))

        a_list = [(b, w) for b in range(NBLK) for w in range(NACH)]
        for step in range(NKB):
            b, w = a_list[step]
            super_step(b, w, step)
        while pending_acc:
            emit_acc(*pending_acc.pop())

        # ---- G = onehot @ S (needs the AllReduced s_sb) ----
        g_ps = auxp.tile([P, 512], f32, tag="aux")
        for b in range(NBLK):
            nc.tensor.matmul(
                g_ps[:, b * C : (b + 1) * C],
                lhsT=ohbt_sb[:, b * P : (b + 1) * P],
                rhs=s_sb[:, 0:C],
                start=True,
                stop=True,
            )
        t2 = p3s.tile([P, NBLK * C], f32, tag="p3")
        t2_blk = t2[:].rearrange("p (b c) -> p b c", c=C)
        nc.vector.tensor_mul(t2_blk, xaug_blk, g_ps[:].rearrange("p (b c) -> p b c", c=C))
        nc.vector.reduce_sum(poss[:], t2_blk, axis=AX.X)
        cnt_ps = auxp.tile([P, 512], f32, tag="aux")
        for b in range(NBLK):
            nc.tensor.matmul(
                cnt_ps[:, b : b + 1],
                lhsT=ohbt_sb[:, b * P : (b + 1) * P],
                rhs=s_sb[:, C : C + 1],
                start=True,
                stop=True,
            )
        nc.vector.tensor_copy(cnt[:], cnt_ps[:, 0:NBLK])

        # ---- tail: combine rowsums ----
        # ACT-side per-block rowsums
        nc.vector.reduce_sum(
            se[:], esum[:].rearrange("p (b w) -> p b w", w=NACH), axis=AX.X
        )
        # DVE-side: drain accum, transpose to row-major, gather
        nc.vector.tensor_copy(acc_sb[:], acc[:])
        tps = auxp.tile([P, 512], bf16, tag="aux")
        for w in range(4):
            nc.tensor.transpose(
                tps[:, w * P : (w + 1) * P],
                acc_sb[:, w * P : (w + 1) * P],
                idn_sb[:],
            )
        # seD[p, b] with b = h*4 + w at tps[p, w*128 + h*64]
        tq = tps[:].rearrange("p (w q r) -> p w q r", w=4, q=2)
        seD_v = seD[:].rearrange("p (h w o) -> p w h o", w=4, o=1)
        nc.vector.tensor_copy(seD_v, tq[:, :, 0:2, 0:1])

        # se_tot = se + KDV * seD
        se_tot = stats.tile([P, NBLK], f32)
        nc.vector.scalar_tensor_tensor(
            out=se_tot[:], in0=seD[:], scalar=KDV, in1=se[:],
            op0=ALU.mult, op1=ALU.add,
        )

        # robust ln via exponent/mantissa split; ln(mantissa) by a DVE
        # cubic (keeps the Ln ACT-table load off the kernel entirely)
        LC = np.polyfit(
            np.cos(np.linspace(0, np.pi, 64)) * 0.5 + 1.5,
            np.log(np.cos(np.linspace(0, np.pi, 64)) * 0.5 + 1.5),
            3,
        )  # c3..c0 for ln(m), m in [1,2]
        fin = stats

        def emit_ln(dst, src, nm):
            sec = fin.tile([P, NBLK], f32, name=f"{nm}_sec")
            nc.vector.tensor_scalar_max(sec[:], src, 1e-37)
            bits = sec[:].bitcast(i32)
            exi = fin.tile([P, NBLK], i32, name=f"{nm}_exi")
            nc.vector.tensor_scalar(
                out=exi[:], in0=bits, scalar1=23, scalar2=None,
                op0=ALU.arith_shift_right,
            )
            exf = fin.tile([P, NBLK], f32, name=f"{nm}_exf")
            nc.vector.tensor_copy(exf[:], exi[:])
            mbits = fin.tile([P, NBLK], i32, name=f"{nm}_mb")
            nc.vector.tensor_scalar(
                out=mbits[:], in0=bits, scalar1=0x007FFFFF, scalar2=0x3F800000,
                op0=ALU.bitwise_and, op1=ALU.bitwise_or,
            )
            m = mbits[:].bitcast(f32)
            t1 = fin.tile([P, NBLK], f32, name=f"{nm}_t1")
            nc.vector.tensor_scalar(
                out=t1[:], in0=m, scalar1=float(LC[0]), scalar2=float(LC[1]),
                op0=ALU.mult, op1=ALU.add,
            )
            t2 = fin.tile([P, NBLK], f32, name=f"{nm}_t2")
            nc.vector.tensor_mul(t2[:], t1[:], m)
            t3 = fin.tile([P, NBLK], f32, name=f"{nm}_t3")
            nc.vector.tensor_scalar(
                out=t3[:], in0=t2[:], scalar1=float(LC[2]), scalar2=None,
                op0=ALU.add,
            )
            t4 = fin.tile([P, NBLK], f32, name=f"{nm}_t4")
            nc.vector.tensor_mul(t4[:], t3[:], m)
            # dst = t4 + c0 + (exf - 127)*ln2
            t5 = fin.tile([P, NBLK], f32, name=f"{nm}_t5")
            nc.vector.tensor_scalar(
                out=t5[:], in0=exf[:], scalar1=-127.0,
                scalar2=float(np.log(2.0)), op0=ALU.add, op1=ALU.mult,
            )
            t6 = fin.tile([P, NBLK], f32, name=f"{nm}_t6")
            nc.vector.tensor_scalar(
                out=t6[:], in0=t4[:], scalar1=float(LC[3]), scalar2=None,
                op0=ALU.add,
            )
            nc.vector.tensor_add(dst, t6[:], t5[:])

        lnse = fin.tile([P, NBLK], f32)
        emit_ln(lnse[:], se_tot[:], "lnse")

        nposc = fin.tile([P, NBLK], f32)
        nc.vector.tensor_scalar(
            out=nposc[:], in0=cnt[:], scalar1=-1.0, scalar2=1.0,
            op0=ALU.add, op1=ALU.max,
        )
        mask = fin.tile([P, NBLK], f32)
        nc.vector.tensor_scalar(
            out=mask[:], in0=cnt[:], scalar1=-1.0, scalar2=1.0,
            op0=ALU.add, op1=ALU.min,
        )
        rc = fin.tile([P, NBLK], f32)
        nc.vector.reciprocal(rc[:], nposc[:])

        pd = fin.tile([P, NBLK], f32)
        nc.vector.tensor_sub(pd[:], poss[:], nrm[:])
        pt = fin.tile([P, NBLK], f32)
        nc.vector.scalar_tensor_tensor(
            out=pt[:], in0=pd[:], scalar=1.0 / TAU, in1=rc[:],
            op0=ALU.mult, op1=ALU.mult,
        )
        peri = fin.tile([P, NBLK], f32)
        nc.vector.scalar_tensor_tensor(
            out=peri[:], in0=lnse[:], scalar=SHIFT, in1=pt[:],
            op0=ALU.add, op1=ALU.subtract,
        )
        perim = fin.tile([P, NBLK], f32)
        nc.vector.tensor_mul(perim[:], peri[:], mask[:])

        lnce = fin.tile([P, NBLK], f32)
        emit_ln(lnce[:], cesum[:], "lnce")
        cec = fin.tile([P, NBLK], f32)
        nc.vector.tensor_sub(cec[:], lnce[:], lgt[:])

        nc.vector.reduce_sum(res[:, 0:1], perim[:], axis=AX.X)
        nc.vector.reduce_sum(res[:, 1:2], cec[:], axis=AX.X)
        nc.sync.dma_start(out_d.ap(), res[:])

        # ---- debug dump ----
        dbg = stats.tile([P, 1600], f32)
        nc.vector.memset(dbg[:], 0.0)
        nc.vector.tensor_copy(dbg[:, 0:512], acc_sb[:])
        nc.vector.tensor_copy(dbg[:, 1024:1032], seD[:])
        nc.vector.tensor_copy(dbg[:, 1032:1040], se[:])
        nc.vector.tensor_copy(dbg[:, 1040:1048], se_tot[:])
        nc.vector.tensor_copy(dbg[:, 1048:1056], poss[:])
        nc.vector.tensor_copy(dbg[:, 1056:1064], cnt[:])
        nc.vector.tensor_copy(dbg[:, 1064:1072], nrm[:])
        nc.vector.tensor_copy(dbg[:, 1072:1080], lgt[:])
        nc.vector.tensor_copy(dbg[:, 1080:1088], cesum[:])
        nc.vector.tensor_copy(dbg[:, 1088:1096], lnse[:])
        nc.vector.tensor_copy(dbg[:, 1096:1104], perim[:])
        nc.sync.dma_start(dbg_d.ap(), dbg[:])

    with tile.TileContext(nc) as tc, ExitStack() as ctx:
        emit(tc, ctx)

    nc.compile()
    return nc


def _get_nc(**kw):
    key = repr(sorted(kw.items()))
    if key not in _CACHE:
        _CACHE[key] = _build(**kw)
    return _CACHE[key]


def _make_in_maps(X, y):
    import ml_dtypes

    bf = ml_dtypes.bfloat16
    X = np.ascontiguousarray(np.asarray(X, dtype=np.float32))
    y = np.asarray(y).astype(np.int64).ravel()
    assert X.shape == (N, C) and y.shape == (N,)

    oh = (y[:, None] == np.arange(C)[None, :]).astype(np.float32)
    xt2 = np.ascontiguousarray((X.T / np.float32(TAU)).astype(bf))
    eyeneg = (np.eye(P) * -1e4).astype(bf)
    ident = np.eye(P).astype(bf)

    in_maps = []
    for r in range(NCORES):
        rows = slice(r * RPC, (r + 1) * RPC)
        xb = X[rows]
        xt2r = np.roll(xt2, -r * RPC, axis=1)
        xaug = np.concatenate([xb, np.ones((RPC, 1), np.float32)], axis=1)
        # per-block layouts [p, b*K+c], contiguous for straight DMA
        xaug_pb = np.ascontiguousarray(
            xaug.reshape(NBLK, P, C + 1).transpose(1, 0, 2).reshape(P, -1)
        ).astype(bf)
        ohb_pb = np.ascontiguousarray(
            oh[rows].reshape(NBLK, P, C).transpose(1, 0, 2).reshape(P, -1)
        ).astype(bf)
        in_maps.append(
            {
                "xt2a": np.ascontiguousarray(xt2r[:, :ACOLS]),
                "xt2b": np.ascontiguousarray(xt2r[:, ACOLS:]),
                "xbt": np.ascontiguousarray(xb.T.astype(bf)),
                "xaug": xaug_pb,
                "ohb": ohb_pb,
                "ohbt": np.ascontiguousarray(oh[rows].T.astype(bf)),
                "eyeneg": eyeneg,
                "ident": ident,
            }
        )
    return in_maps


def run(input, target, trace=False, **build_kw):
    """Run the device kernel; returns (loss_scalar, BassKernelResults)."""
    from concourse.bass_utils import run_bass_kernel_spmd

    nc = _get_nc(**build_kw)
    in_maps = _make_in_maps(input, target)
    res = run_bass_kernel_spmd(
        nc, in_maps, core_ids=list(range(NCORES)), trace=trace
    )
    sc = 0.0
    ce = 0.0
    for core_out in res.results:
        o = core_out["out"].astype(np.float64)
        sc += o[:, 0].sum()
        ce += o[:, 1].sum()
    loss = (1.0 - LMBD) * (ce / N) + LMBD * sc
    return np.array(loss, dtype=np.float32), res


def kernel(input, target):
    loss, _ = run(input, target, trace=False)
    return loss


# revision 19
# speedup vs baseline: 1.0921x; 1.0921x over previous
"""Fused CE + supervised-contrastive loss on 8 Trainium2 NeuronCores.

Math (reference semantics):
  ce   = -mean_i log_softmax(input)[i, y_i]
  sim  = (X @ X.T) / tau, diag excluded
  lse_i = logsumexp_{k!=i} sim[i,k]
  possum_i = (x_i . S_{y_i} - ||x_i||^2)/tau, S_c = sum_{k: y_k=c} x_k
  per_i = lse_i - possum_i/n_pos_i  (0 if n_pos_i == 0)
  loss = (1-lmbd)*ce + lmbd * sum_i per_i

Distribution: each core owns 1024 rows, streams all 8192 columns. The only
O(N^2) work is the row-wise sum of exp(sim); it is split across TWO engines
working in parallel on a shared 3-slot PSUM rotation (6 banks):

  * ACT path (cols 0..ACOLS): row-major sim chunks [128 rows, 1024 cols].
    ACT exp with bias=-100 and fused accum_out row-sums (esum). The diagonal
    (always in local cols [0,1024) thanks to the per-core xt2 rotation) is
    killed pre-exp by a diag(-1e4) accumulate-matmul, as in the classic
    flash pattern.
  * DVE path (cols ACOLS..8192): TRANSPOSED sim chunks [128 cols, 1024 rows]
    (lhsT = xt2 column block, rhs = xbt). DVE computes a one-instruction
    Schraudolph fast-exp: bits16 = trunc(max(sim,0) * 128/ln2) as int16,
    bitcast to bf16 == e^(sim - 127*ln2) * rho, rho in [1, 1.086]
    (measured mean 1.0410 on HW, folded into KDV below). The PE then
    row-sums those bf16 tiles with ones-lhsT matmuls into a persistent
    PSUM accumulator (4 sub-accumulators at partitions {0,32,64,96} for
    the 2 K-halves x 2 row-halves), giving rowsums with rows in the FREE
    dim; a 4x PE-transpose at the tail converts to row-major [128, 8].
    The max(.,0) clamp is mandatory: the int16 convert WRAPS on negative
    (clamped terms contribute exactly +0.0, an error < e^-88 per term).

  PE lanes: ACT fills run on array rows 0-63 (lo operand copies), DVE fills
  on rows 64-127 (hi copies) -- concurrent via tile_position row groups.
  The ones-reduces split K=64 lo/hi the same way.

  Combine: se_i = esum_i + KDV * seD_i with KDV = e^(127*ln2 - 100)/1.0410
  (shift alignment + measured Schraudolph mean bias).

Class sums S (and class counts, S's last column) are computed per-core and
AllReduce'd first thing -- the collective's ncfw/barrier latency overlaps
the exp phase; everything S-dependent (G gather, possum, counts) sits at
the tail.

PSUM (8 banks): 3x [128,1024] f32 stream slots (6) + rowsum accum [97,512]
(1) + aux rotation S->G->cnt->transpose (1).
"""

import math

import numpy as np

N, C = 8192, 64
NCORES = 8
RPC = N // NCORES          # rows per core (1024)
P = 128                    # partitions per row-block
NBLK = RPC // P            # 8 row blocks per core
TAU = 0.5
LMBD = 0.5
SHIFT = 100.0              # ACT-path logsumexp shift
ACOLS = 4096               # ACT row-major columns per core
DCOLS = N - ACOLS          # DVE transposed columns per core
AW = 1024                  # ACT chunk width
NACH = ACOLS // AW         # ACT chunks per block (4)
NKB = DCOLS // P           # DVE column blocks (32)
AEXP = 128.0 / math.log(2.0)          # fast-exp scale (184.6646)
RHO = 1.0410                          # measured mean Schraudolph ratio (HW)
KDV = math.exp(127.0 * math.log(2.0) - SHIFT) / RHO

_CACHE = {}


def _build():
    from contextlib import ExitStack

    import concourse.bass as bass
    import concourse.tile as tile
    from concourse import bacc, mybir

    f32 = mybir.dt.float32
    i16 = mybir.dt.int16
    i32 = mybir.dt.int32
    bf16 = mybir.dt.bfloat16
    AF = mybir.ActivationFunctionType
    ALU = mybir.AluOpType
    AX = mybir.AxisListType

    nc = bacc.Bacc(
        "TRN2",
        target_bir_lowering=False,
        debug=False,
        num_devices=NCORES,
    )

    xt2a_d = nc.dram_tensor("xt2a", [C, ACOLS], bf16, kind="ExternalInput")
    xt2b_d = nc.dram_tensor("xt2b", [C, DCOLS], bf16, kind="ExternalInput")
    xbt_d = nc.dram_tensor("xbt", [C, RPC], bf16, kind="ExternalInput")
    xaug_d = nc.dram_tensor("xaug", [P, NBLK * (C + 1)], bf16, kind="ExternalInput")
    ohb_d = nc.dram_tensor("ohb", [P, NBLK * C], bf16, kind="ExternalInput")
    ohbt_d = nc.dram_tensor("ohbt", [C, RPC], bf16, kind="ExternalInput")
    eye_d = nc.dram_tensor("eyeneg", [P, P], bf16, kind="ExternalInput")
    idn_d = nc.dram_tensor("ident", [P, P], bf16, kind="ExternalInput")
    out_d = nc.dram_tensor("out", [P, 2], f32, kind="ExternalOutput")

    def emit(tc, ctx):
        const = ctx.enter_context(tc.tile_pool(name="const", bufs=1))
        dram = ctx.enter_context(tc.tile_pool(name="dram", bufs=1, space="DRAM"))
        strm = ctx.enter_context(tc.tile_pool(name="strm", bufs=3, space="PSUM"))
        accp = ctx.enter_context(tc.tile_pool(name="accp", bufs=1, space="PSUM"))
        auxp = ctx.enter_context(tc.tile_pool(name="auxp", bufs=1, space="PSUM"))
        scrp = ctx.enter_context(tc.tile_pool(name="scrp", bufs=2))
        ep = ctx.enter_context(tc.tile_pool(name="ep", bufs=3))
        p3s = ctx.enter_context(tc.tile_pool(name="p3s", bufs=2))
        stats = ctx.enter_context(tc.tile_pool(name="stats", bufs=1))

        # ---- input DMAs: S-path operands lead the sync queue so every
        # core reaches the AllReduce within ~2us (the collective barrier
        # waits for the slowest core); big xt2 streams behind / parallel ----
        xaug_sb = const.tile([P, NBLK * (C + 1)], bf16)
        nc.sync.dma_start(xaug_sb[:], xaug_d.ap())
        ohb_sb = const.tile([P, NBLK * C], bf16)
        nc.sync.dma_start(ohb_sb[:], ohb_d.ap())

        xbt_sb = const.tile([P, RPC], bf16)
        nc.sync.dma_start(xbt_sb[0:C, :], xbt_d.ap())
        nc.sync.dma_start(xbt_sb[C:P, :], xbt_d.ap())
        xt2b_sb = const.tile([P, DCOLS], bf16)
        nc.gpsimd.dma_start(xt2b_sb[C:P, 0:AW], xt2b_d.ap()[:, 0:AW])
        xt2a_sb = const.tile([P, ACOLS], bf16)
        nc.sync.dma_start(xt2a_sb[0:C, 0:AW], xt2a_d.ap()[:, 0:AW])
        eye_sb = const.tile([P, P], bf16)
        nc.sync.dma_start(eye_sb[:], eye_d.ap())
        idn_sb = const.tile([P, P], bf16)
        nc.sync.dma_start(idn_sb[:], idn_d.ap())
        nc.sync.dma_start(xt2a_sb[0:C, AW:ACOLS], xt2a_d.ap()[:, AW:ACOLS])
        nc.gpsimd.dma_start(xt2b_sb[C:P, AW:DCOLS], xt2b_d.ap()[:, AW:DCOLS])
        ohbt_sb = const.tile([C, RPC], bf16)
        nc.gpsimd.dma_start(ohbt_sb[:], ohbt_d.ap())

        # ---- persistent small tiles ----
        nshift = stats.tile([P, 1], f32)
        nc.vector.memset(nshift[:], -SHIFT)
        # masked-ones lhsT for the rowsum reduce: mh0 selects out rows 0-63
        # (row-half 0), mh1 rows 64-127 (half 1); both sum all 128 k
        masks = stats.tile([P, 2 * P], bf16)
        nc.vector.memset(masks[:, 0:C], 1.0)
        nc.vector.memset(masks[:, C:P], 0.0)
        nc.vector.memset(masks[:, P : P + C], 0.0)
        nc.vector.memset(masks[:, P + C : 2 * P], 1.0)
        esum = stats.tile([P, NBLK * NACH], f32)
        acc_sb = stats.tile([P, 512], bf16)
        se = stats.tile([P, NBLK], f32)
        seD = stats.tile([P, NBLK], f32)
        nrm = stats.tile([P, NBLK], f32)
        poss = stats.tile([P, NBLK], f32)
        cnt = stats.tile([P, NBLK], f32)
        lgt = stats.tile([P, NBLK], f32)
        cesum = stats.tile([P, NBLK], f32)
        s_loc = stats.tile([C, C + 1], f32)
        s_sb = stats.tile([C, C + 1], bf16)
        res = stats.tile([P, 2], f32)

        # rowsum accumulator: rows 0-63 hold row-half 0 sums (redundant
        # copies), rows 64-127 half 1; free = row-within-half
        acc = accp.tile([P, 512], f32)

        # ---- class sums S + AllReduce, first compute on the device ----
        s_ps = auxp.tile([P, 512], f32, tag="aux")
        s_view = s_ps[:C, : C + 1]
        for b in range(NBLK):
            nc.tensor.matmul(
                s_view,
                lhsT=ohb_sb[:, b * C : (b + 1) * C],
                rhs=xaug_sb[:, b * (C + 1) : (b + 1) * (C + 1)],
                start=(b == 0),
                stop=(b == NBLK - 1),
            )
        nc.vector.tensor_copy(s_loc[:], s_view)
        s_in = dram.tile([C, C + 1], f32)
        s_out = dram.tile([C, C + 1], f32)
        nc.sync.dma_start(s_in[:], s_loc[:])
        nc.gpsimd.collective_compute(
            "AllReduce",
            mybir.AluOpType.add,
            replica_groups=[list(range(NCORES))],
            ins=[s_in.opt()],
            outs=[s_out.opt()],
        )

        # ---- early DVE small-ops (fit into stream slack) ----
        xaug_blk = xaug_sb[:].rearrange("p (b c) -> p b c", c=C + 1)[:, :, 0:C]
        ohb_blk = ohb_sb[:].rearrange("p (b c) -> p b c", c=C)
        t0 = p3s.tile([P, NBLK * C], f32, tag="p3")
        t0_blk = t0[:].rearrange("p (b c) -> p b c", c=C)
        nc.vector.tensor_mul(t0_blk, xaug_blk, xaug_blk)
        nc.vector.reduce_sum(nrm[:], t0_blk, axis=AX.X)
        t1 = p3s.tile([P, NBLK * C], f32, tag="p3")
        t1_blk = t1[:].rearrange("p (b c) -> p b c", c=C)
        nc.vector.tensor_mul(t1_blk, xaug_blk, ohb_blk)
        nc.vector.reduce_sum(lgt[:], t1_blk, axis=AX.X)
        # CE denominators: one batched exp (incl. the ones column; skipped in
        # the reduce via a strided AP)
        cescr = p3s.tile([P, NBLK * (C + 1)], f32, tag="ce")
        nc.scalar.activation(cescr[:], xaug_sb[:], AF.Exp)
        nc.vector.reduce_sum(
            cesum[:],
            cescr[:].rearrange("p (b c) -> p b c", c=C + 1)[:, :, 0:C],
            axis=AX.X,
        )

        # ---- main interleaved exp stream ----
        # Each super-step processes one ACT chunk (row-major, lo operands,
        # PE row-group 0) and one DVE chunk (transposed, hi operands, PE
        # row-group 64) with fills interleaved so adjacent matmuls target
        # different row groups and run concurrently.  The masked-ones
        # rowsum-reduce matmuls of the PREVIOUS DVE chunk are emitted after
        # this step's fills so the in-order PE queue never waits on DVE.
        pending_acc = []

        def emit_acc(kb, eb):
            for h in range(2):
                nc.tensor.matmul(
                    acc[:, 0:512],
                    lhsT=masks[:, h * P : (h + 1) * P],
                    rhs=eb[:, h * 512 : (h + 1) * 512],
                    start=(kb == 0 and h == 0),
                    stop=(kb == NKB - 1 and h == 1),
                    skip_group_check=True,
                )

        def super_step(b, w, kb):
            ps_a = strm.tile([P, AW], f32, tag="s")
            ps_d = strm.tile([P, AW], f32, tag="s")
            for h in range(2):
                nc.tensor.matmul(
                    ps_d[:, h * 512 : (h + 1) * 512],
                    lhsT=xt2b_sb[C:P, kb * P : (kb + 1) * P],
                    rhs=xbt_sb[C:P, h * 512 : (h + 1) * 512],
                    start=True,
                    stop=True,
                )
                nc.tensor.matmul(
                    ps_a[:, h * 512 : (h + 1) * 512],
                    lhsT=xbt_sb[0:C, b * P : (b + 1) * P],
                    rhs=xt2a_sb[0:C, w * AW + h * 512 : w * AW + (h + 1) * 512],
                    start=True,
                    stop=True,
                )
            if w == 0:
                # kill self-similarity (local col b*128+p) pre-exp
                nc.tensor.matmul(
                    ps_a[:, b * P : (b + 1) * P],
                    lhsT=idn_sb[:],
                    rhs=eye_sb[:],
                    start=False,
                    stop=True,
                    skip_group_check=True,
                )
            if pending_acc:
                emit_acc(*pending_acc.pop())
            scr = scrp.tile([P, AW], bf16, tag="scr")
            idx = b * NACH + w
            nc.scalar.activation(
                scr[:], ps_a[:], AF.Exp, bias=nshift[:],
                accum_out=esum[:, idx : idx + 1],
            )
            et = ep.tile([P, AW], i16, tag="E")
            nc.vector.tensor_scalar(
                out=et[:], in0=ps_d[:], scalar1=0.0, scalar2=AEXP,
                op0=ALU.max, op1=ALU.mult,
            )
            pending_acc.append((kb, et[:].bitcast(bf16)))

        a_list = [(b, w) for b in range(NBLK) for w in range(NACH)]
        for step in range(NKB):
            b, w = a_list[step]
            super_step(b, w, step)
        while pending_acc:
            emit_acc(*pending_acc.pop())

        # ---- G = onehot @ S (needs the AllReduced s_sb) ----
        # (the readback DMA is emitted HERE, after the whole stream, so its
        # wait on the collective never blocks stream bookkeeping queued
        # behind it on the gpsimd engine)
        nc.gpsimd.dma_start(s_sb[:], s_out[:])  # casts f32 -> bf16
        g_ps = auxp.tile([P, 512], f32, tag="aux")
        for b in range(NBLK):
            nc.tensor.matmul(
                g_ps[:, b * C : (b + 1) * C],
                lhsT=ohbt_sb[:, b * P : (b + 1) * P],
                rhs=s_sb[:, 0:C],
                start=True,
                stop=True,
            )
        t2 = p3s.tile([P, NBLK * C], f32, tag="p3")
        t2_blk = t2[:].rearrange("p (b c) -> p b c", c=C)
        nc.vector.tensor_mul(t2_blk, xaug_blk, g_ps[:].rearrange("p (b c) -> p b c", c=C))
        nc.vector.reduce_sum(poss[:], t2_blk, axis=AX.X)
        cnt_ps = auxp.tile([P, 512], f32, tag="aux")
        for b in range(NBLK):
            nc.tensor.matmul(
                cnt_ps[:, b : b + 1],
                lhsT=ohbt_sb[:, b * P : (b + 1) * P],
                rhs=s_sb[:, C : C + 1],
                start=True,
                stop=True,
            )
        nc.vector.tensor_copy(cnt[:], cnt_ps[:, 0:NBLK])

        # ---- tail: combine rowsums ----
        # ACT-side per-block rowsums
        nc.vector.reduce_sum(
            se[:], esum[:].rearrange("p (b w) -> p b w", w=NACH), axis=AX.X
        )
        # DVE-side: drain accum, transpose to row-major, gather
        nc.vector.tensor_copy(acc_sb[:], acc[:])
        tps = auxp.tile([P, 512], bf16, tag="aux")
        for w in range(4):
            nc.tensor.transpose(
                tps[:, w * P : (w + 1) * P],
                acc_sb[:, w * P : (w + 1) * P],
                idn_sb[:],
            )
        # seD[p, b] with b = h*4 + w at tps[p, w*128 + h*64]
        tq = tps[:].rearrange("p (w q r) -> p w q r", w=4, q=2)
        seD_v = seD[:].rearrange("p (h w o) -> p w h o", w=4, o=1)
        nc.vector.tensor_copy(seD_v, tq[:, :, 0:2, 0:1])

        # se_tot = se + KDV * seD
        se_tot = stats.tile([P, NBLK], f32)
        nc.vector.scalar_tensor_tensor(
            out=se_tot[:], in0=seD[:], scalar=KDV, in1=se[:],
            op0=ALU.mult, op1=ALU.add,
        )

        # robust ln via exponent/mantissa split; ln(mantissa) by a DVE
        # cubic (keeps the Ln ACT-table load off the kernel entirely)
        LC = np.polyfit(
            np.cos(np.linspace(0, np.pi, 64)) * 0.5 + 1.5,
            np.log(np.cos(np.linspace(0, np.pi, 64)) * 0.5 + 1.5),
            3,
        )  # c3..c0 for ln(m), m in [1,2]
        fin = stats

        def emit_ln(dst, src, nm):
            sec = fin.tile([P, NBLK], f32, name=f"{nm}_sec")
            nc.vector.tensor_scalar_max(sec[:], src, 1e-37)
            bits = sec[:].bitcast(i32)
            exi = fin.tile([P, NBLK], i32, name=f"{nm}_exi")
            nc.vector.tensor_scalar(
                out=exi[:], in0=bits, scalar1=23, scalar2=None,
                op0=ALU.arith_shift_right,
            )
            exf = fin.tile([P, NBLK], f32, name=f"{nm}_exf")
            nc.vector.tensor_copy(exf[:], exi[:])
            mbits = fin.tile([P, NBLK], i32, name=f"{nm}_mb")
            nc.vector.tensor_scalar(
                out=mbits[:], in0=bits, scalar1=0x007FFFFF, scalar2=0x3F800000,
                op0=ALU.bitwise_and, op1=ALU.bitwise_or,
            )
            m = mbits[:].bitcast(f32)
            t1 = fin.tile([P, NBLK], f32, name=f"{nm}_t1")
            nc.vector.tensor_scalar(
                out=t1[:], in0=m, scalar1=float(LC[0]), scalar2=float(LC[1]),
                op0=ALU.mult, op1=ALU.add,
            )
            t2 = fin.tile([P, NBLK], f32, name=f"{nm}_t2")
            nc.vector.tensor_mul(t2[:], t1[:], m)
            t3 = fin.tile([P, NBLK], f32, name=f"{nm}_t3")
            nc.vector.tensor_scalar(
                out=t3[:], in0=t2[:], scalar1=float(LC[2]), scalar2=None,
                op0=ALU.add,
            )
            t4 = fin.tile([P, NBLK], f32, name=f"{nm}_t4")
            nc.vector.tensor_mul(t4[:], t3[:], m)
            # dst = t4 + c0 + (exf - 127)*ln2
            t5 = fin.tile([P, NBLK], f32, name=f"{nm}_t5")
            nc.vector.tensor_scalar(
                out=t5[:], in0=exf[:], scalar1=-127.0,
                scalar2=float(np.log(2.0)), op0=ALU.add, op1=ALU.mult,
            )
            t6 = fin.tile([P, NBLK], f32, name=f"{nm}_t6")
            nc.vector.tensor_scalar(
                out=t6[:], in0=t4[:], scalar1=float(LC[3]), scalar2=None,
                op0=ALU.add,
            )
            nc.vector.tensor_add(dst, t6[:], t5[:])

        lnse = fin.tile([P, NBLK], f32)
        emit_ln(lnse[:], se_tot[:], "lnse")

        nposc = fin.tile([P, NBLK], f32)
        nc.vector.tensor_scalar(
            out=nposc[:], in0=cnt[:], scalar1=-1.0, scalar2=1.0,
            op0=ALU.add, op1=ALU.max,
        )
        mask = fin.tile([P, NBLK], f32)
        nc.vector.tensor_scalar(
            out=mask[:], in0=cnt[:], scalar1=-1.0, scalar2=1.0,
            op0=ALU.add, op1=ALU.min,
        )
        rc = fin.tile([P, NBLK], f32)
        nc.vector.reciprocal(rc[:], nposc[:])

        pd = fin.tile([P, NBLK], f32)
        nc.vector.tensor_sub(pd[:], poss[:], nrm[:])
        pt = fin.tile([P, NBLK], f32)
        nc.vector.scalar_tensor_tensor(
            out=pt[:], in0=pd[:], scalar=1.0 / TAU, in1=rc[:],
            op0=ALU.mult, op1=ALU.mult,
        )
        peri = fin.tile([P, NBLK], f32)
        nc.vector.scalar_tensor_tensor(
            out=peri[:], in0=lnse[:], scalar=SHIFT, in1=pt[:],
            op0=ALU.add, op1=ALU.subtract,
        )
        perim = fin.tile([P, NBLK], f32)
        nc.vector.tensor_mul(perim[:], peri[:], mask[:])

        lnce = fin.tile([P, NBLK], f32)
        emit_ln(lnce[:], cesum[:], "lnce")
        cec = fin.tile([P, NBLK], f32)
        nc.vector.tensor_sub(cec[:], lnce[:], lgt[:])

        nc.vector.reduce_sum(res[:, 0:1], perim[:], axis=AX.X)
        nc.vector.reduce_sum(res[:, 1:2], cec[:], axis=AX.X)
        nc.sync.dma_start(out_d.ap(), res[:])

    with tile.TileContext(nc) as tc, ExitStack() as ctx:
        emit(tc, ctx)

    nc.compile()
    return nc


def _get_nc(**kw):
    key = repr(sorted(kw.items()))
    if key not in _CACHE:
        _CACHE[key] = _build(**kw)
    return _CACHE[key]


def _make_in_maps(X, y):
    import ml_dtypes

    bf = ml_dtypes.bfloat16
    X = np.ascontiguousarray(np.asarray(X, dtype=np.float32))
    y = np.asarray(y).astype(np.int64).ravel()
    assert X.shape == (N, C) and y.shape == (N,)

    oh = (y[:, None] == np.arange(C)[None, :]).astype(np.float32)
    xt2 = np.ascontiguousarray((X.T / np.float32(TAU)).astype(bf))
    eyeneg = (np.eye(P) * -1e4).astype(bf)
    ident = np.eye(P).astype(bf)

    in_maps = []
    for r in range(NCORES):
        rows = slice(r * RPC, (r + 1) * RPC)
        xb = X[rows]
        xt2r = np.roll(xt2, -r * RPC, axis=1)
        xaug = np.concatenate([xb, np.ones((RPC, 1), np.float32)], axis=1)
        # per-block layouts [p, b*K+c], contiguous for straight DMA
        xaug_pb = np.ascontiguousarray(
            xaug.reshape(NBLK, P, C + 1).transpose(1, 0, 2).reshape(P, -1)
        ).astype(bf)
        ohb_pb = np.ascontiguousarray(
            oh[rows].reshape(NBLK, P, C).transpose(1, 0, 2).reshape(P, -1)
        ).astype(bf)
        in_maps.append(
            {
                "xt2a": np.ascontiguousarray(xt2r[:, :ACOLS]),
                "xt2b": np.ascontiguousarray(xt2r[:, ACOLS:]),
                "xbt": np.ascontiguousarray(xb.T.astype(bf)),
                "xaug": xaug_pb,
                "ohb": ohb_pb,
                "ohbt": np.ascontiguousarray(oh[rows].T.astype(bf)),
                "eyeneg": eyeneg,
                "ident": ident,
            }
        )
    return in_maps


def run(input, target, trace=False, **build_kw):
    """Run the device kernel; returns (loss_scalar, BassKernelResults)."""
    from concourse.bass_utils import run_bass_kernel_spmd

    nc = _get_nc(**build_kw)
    in_maps = _make_in_maps(input, target)
    res = run_bass_kernel_spmd(
        nc, in_maps, core_ids=list(range(NCORES)), trace=trace
    )
    sc = 0.0
    ce = 0.0
    for core_out in res.results:
        o = core_out["out"].astype(np.float64)
        sc += o[:, 0].sum()
        ce += o[:, 1].sum()
    loss = (1.0 - LMBD) * (ce / N) + LMBD * sc
    return np.array(loss, dtype=np.float32), res


def kernel(input, target):
    loss, _ = run(input, target, trace=False)
    return loss


# revision 20
# speedup vs baseline: 1.2511x; 1.1455x over previous
"""Fused CE + supervised-contrastive loss on 8 Trainium2 NeuronCores.

Math (reference semantics):
  ce   = -mean_i log_softmax(input)[i, y_i]
  sim  = (X @ X.T) / tau, diag excluded
  lse_i = logsumexp_{k!=i} sim[i,k]
  possum_i = (x_i . S_{y_i} - ||x_i||^2)/tau, S_c = sum_{k: y_k=c} x_k
  per_i = lse_i - possum_i/n_pos_i  (0 if n_pos_i == 0)
  loss = (1-lmbd)*ce + lmbd * sum_i per_i

Distribution: each core owns 1024 rows, streams all 8192 columns. The only
O(N^2) work is the row-wise sum of exp(sim); it is split across TWO engines
working in parallel on a shared 3-slot PSUM rotation (6 banks):

  * ACT path (cols 0..ACOLS): row-major sim chunks [128 rows, 1024 cols].
    ACT exp with bias=-100 and fused accum_out row-sums (esum). The diagonal
    (always in local cols [0,1024) thanks to the per-core xt2 rotation) is
    killed pre-exp by a diag(-1e4) accumulate-matmul, as in the classic
    flash pattern.
  * DVE path (cols ACOLS..8192): TRANSPOSED sim chunks [128 cols, 1024 rows]
    (lhsT = xt2 column block, rhs = xbt). DVE computes a one-instruction
    Schraudolph fast-exp: bits16 = trunc(max(sim,0) * 128/ln2) as int16,
    bitcast to bf16 == e^(sim - 127*ln2) * rho, rho in [1, 1.086]
    (measured mean 1.0410 on HW, folded into KDV below). The PE then
    row-sums those bf16 tiles with ones-lhsT matmuls into a persistent
    PSUM accumulator (4 sub-accumulators at partitions {0,32,64,96} for
    the 2 K-halves x 2 row-halves), giving rowsums with rows in the FREE
    dim; a 4x PE-transpose at the tail converts to row-major [128, 8].
    The max(.,0) clamp is mandatory: the int16 convert WRAPS on negative
    (clamped terms contribute exactly +0.0, an error < e^-88 per term).

  PE lanes: ACT fills run on array rows 0-63 (lo operand copies), DVE fills
  on rows 64-127 (hi copies) -- concurrent via tile_position row groups.
  The ones-reduces split K=64 lo/hi the same way.

  Combine: se_i = esum_i + KDV * seD_i with KDV = e^(127*ln2 - 100)/1.0410
  (shift alignment + measured Schraudolph mean bias).

Class sums S (and class counts, S's last column) are computed per-core and
AllReduce'd first thing -- the collective's ncfw/barrier latency overlaps
the exp phase; everything S-dependent (G gather, possum, counts) sits at
the tail.

PSUM (8 banks): 3x [128,1024] f32 stream slots (6) + rowsum accum [97,512]
(1) + aux rotation S->G->cnt->transpose (1).
"""

import math

import numpy as np

N, C = 8192, 64
NCORES = 8
RPC = N // NCORES          # rows per core (1024)
P = 128                    # partitions per row-block
NBLK = RPC // P            # 8 row blocks per core
TAU = 0.5
LMBD = 0.5
SHIFT = 100.0              # ACT-path logsumexp shift
ACOLS = 4096               # ACT row-major columns per core
DCOLS = N - ACOLS          # DVE transposed columns per core
AW = 1024                  # ACT chunk width
NACH = ACOLS // AW         # ACT chunks per block (4)
NKB = DCOLS // P           # DVE column blocks (32)
AEXP = 128.0 / math.log(2.0)          # fast-exp scale (184.6646)
RHO = 1.0410                          # measured mean Schraudolph ratio (HW)
KDV = math.exp(127.0 * math.log(2.0) - SHIFT) / RHO

_CACHE = {}


def _build():
    from contextlib import ExitStack

    import concourse.bass as bass
    import concourse.tile as tile
    from concourse import bacc, mybir

    f32 = mybir.dt.float32
    i16 = mybir.dt.int16
    i32 = mybir.dt.int32
    bf16 = mybir.dt.bfloat16
    AF = mybir.ActivationFunctionType
    ALU = mybir.AluOpType
    AX = mybir.AxisListType

    nc = bacc.Bacc(
        "TRN2",
        target_bir_lowering=False,
        debug=False,
        num_devices=NCORES,
    )

    xt2a_d = nc.dram_tensor("xt2a", [C, ACOLS], bf16, kind="ExternalInput")
    xt2b_d = nc.dram_tensor("xt2b", [C, DCOLS], bf16, kind="ExternalInput")
    xbt_d = nc.dram_tensor("xbt", [C, RPC], bf16, kind="ExternalInput")
    xaug_d = nc.dram_tensor("xaug", [P, NBLK * (C + 1)], bf16, kind="ExternalInput")
    ohb_d = nc.dram_tensor("ohb", [P, NBLK * C], bf16, kind="ExternalInput")
    ohbt_d = nc.dram_tensor("ohbt", [C, RPC], bf16, kind="ExternalInput")
    eye_d = nc.dram_tensor("eyeneg", [P, P], bf16, kind="ExternalInput")
    idn_d = nc.dram_tensor("ident", [P, P], bf16, kind="ExternalInput")
    out_d = nc.dram_tensor("out", [P, 2], f32, kind="ExternalOutput")

    def emit(tc, ctx):
        const = ctx.enter_context(tc.tile_pool(name="const", bufs=1))
        dram = ctx.enter_context(tc.tile_pool(name="dram", bufs=1, space="DRAM"))
        strm = ctx.enter_context(tc.tile_pool(name="strm", bufs=3, space="PSUM"))
        accp = ctx.enter_context(tc.tile_pool(name="accp", bufs=1, space="PSUM"))
        auxp = ctx.enter_context(tc.tile_pool(name="auxp", bufs=1, space="PSUM"))
        scrp = ctx.enter_context(tc.tile_pool(name="scrp", bufs=2))
        ep = ctx.enter_context(tc.tile_pool(name="ep", bufs=3))
        p3s = ctx.enter_context(tc.tile_pool(name="p3s", bufs=2))
        stats = ctx.enter_context(tc.tile_pool(name="stats", bufs=1))

        # ---- input DMAs: S-path operands lead the sync queue so every
        # core reaches the AllReduce within ~2us (the collective barrier
        # waits for the slowest core); big xt2 streams behind / parallel ----
        xaug_sb = const.tile([P, NBLK * (C + 1)], bf16)
        nc.sync.dma_start(xaug_sb[:], xaug_d.ap())
        ohb_sb = const.tile([P, NBLK * C], bf16)
        nc.sync.dma_start(ohb_sb[:], ohb_d.ap())

        xbt_sb = const.tile([P, RPC], bf16)
        nc.sync.dma_start(xbt_sb[0:C, :], xbt_d.ap())
        nc.sync.dma_start(xbt_sb[C:P, :], xbt_d.ap())
        xt2b_sb = const.tile([P, DCOLS], bf16)
        nc.gpsimd.dma_start(xt2b_sb[C:P, 0:AW], xt2b_d.ap()[:, 0:AW])
        xt2a_sb = const.tile([P, ACOLS], bf16)
        nc.sync.dma_start(xt2a_sb[0:C, 0:AW], xt2a_d.ap()[:, 0:AW])
        eye_sb = const.tile([P, P], bf16)
        nc.sync.dma_start(eye_sb[:], eye_d.ap())
        idn_sb = const.tile([P, P], bf16)
        nc.sync.dma_start(idn_sb[:], idn_d.ap())
        nc.sync.dma_start(xt2a_sb[0:C, AW:ACOLS], xt2a_d.ap()[:, AW:ACOLS])
        nc.gpsimd.dma_start(xt2b_sb[C:P, AW:DCOLS], xt2b_d.ap()[:, AW:DCOLS])
        ohbt_sb = const.tile([C, RPC], bf16)
        nc.gpsimd.dma_start(ohbt_sb[:], ohbt_d.ap())

        # ---- persistent small tiles ----
        nshift = stats.tile([P, 1], f32)
        nc.vector.memset(nshift[:], -SHIFT)
        # masked-ones lhsT for the rowsum reduce: mh0 selects out rows 0-63
        # (row-half 0), mh1 rows 64-127 (half 1); both sum all 128 k
        masks = stats.tile([P, 2 * P], bf16)
        nc.vector.memset(masks[:, 0:C], 1.0)
        nc.vector.memset(masks[:, C:P], 0.0)
        nc.vector.memset(masks[:, P : P + C], 0.0)
        nc.vector.memset(masks[:, P + C : 2 * P], 1.0)
        zmask = stats.tile([P, P], bf16)
        nc.vector.memset(zmask[:], 0.0)
        esum = stats.tile([P, NBLK * NACH], f32)
        acc_sb = stats.tile([P, 512], bf16)
        se = stats.tile([P, NBLK], f32)
        seD = stats.tile([P, NBLK], f32)
        nrm = stats.tile([P, NBLK], f32)
        poss = stats.tile([P, NBLK], f32)
        cnt = stats.tile([P, NBLK], f32)
        lgt = stats.tile([P, NBLK], f32)
        cesum = stats.tile([P, NBLK], f32)
        s_loc = stats.tile([C, C + 1], f32)
        s_sb = stats.tile([C, C + 1], bf16)
        res = stats.tile([P, 2], f32)

        # rowsum accumulator: rows 0-63 hold row-half 0 sums (redundant
        # copies), rows 64-127 half 1; free = row-within-half
        acc = accp.tile([P, 512], f32)

        # ---- class sums S + AllReduce, first compute on the device ----
        s_ps = auxp.tile([P, 512], f32, tag="aux")
        s_view = s_ps[:C, : C + 1]
        for b in range(NBLK):
            nc.tensor.matmul(
                s_view,
                lhsT=ohb_sb[:, b * C : (b + 1) * C],
                rhs=xaug_sb[:, b * (C + 1) : (b + 1) * (C + 1)],
                start=(b == 0),
                stop=(b == NBLK - 1),
            )
        nc.vector.tensor_copy(s_loc[:], s_view)
        s_in = dram.tile([C, C + 1], f32)
        s_out = dram.tile([C, C + 1], f32)
        nc.sync.dma_start(s_in[:], s_loc[:])
        nc.gpsimd.collective_compute(
            "AllReduce",
            mybir.AluOpType.add,
            replica_groups=[list(range(NCORES))],
            ins=[s_in.opt()],
            outs=[s_out.opt()],
        )

        # ---- early DVE small-ops (fit into stream slack) ----
        xaug_blk = xaug_sb[:].rearrange("p (b c) -> p b c", c=C + 1)[:, :, 0:C]
        ohb_blk = ohb_sb[:].rearrange("p (b c) -> p b c", c=C)
        t0 = p3s.tile([P, NBLK * C], f32, tag="p3")
        t0_blk = t0[:].rearrange("p (b c) -> p b c", c=C)
        nc.vector.tensor_mul(t0_blk, xaug_blk, xaug_blk)
        nc.vector.reduce_sum(nrm[:], t0_blk, axis=AX.X)
        t1 = p3s.tile([P, NBLK * C], f32, tag="p3")
        t1_blk = t1[:].rearrange("p (b c) -> p b c", c=C)
        nc.vector.tensor_mul(t1_blk, xaug_blk, ohb_blk)
        nc.vector.reduce_sum(lgt[:], t1_blk, axis=AX.X)
        # CE denominators: one batched exp (incl. the ones column; skipped in
        # the reduce via a strided AP)
        cescr = p3s.tile([P, NBLK * (C + 1)], f32, tag="ce")
        nc.scalar.activation(cescr[:], xaug_sb[:], AF.Exp)
        nc.vector.reduce_sum(
            cesum[:],
            cescr[:].rearrange("p (b c) -> p b c", c=C + 1)[:, :, 0:C],
            axis=AX.X,
        )

        # ---- main interleaved exp stream ----
        # Each super-step processes one ACT chunk (row-major, lo operands,
        # PE row-group 0) and one DVE chunk (transposed, hi operands, PE
        # row-group 64) with fills interleaved so adjacent matmuls target
        # different row groups and run concurrently.  The masked-ones
        # rowsum-reduce matmuls of the PREVIOUS DVE chunk are emitted after
        # this step's fills so the in-order PE queue never waits on DVE.
        pending_acc = []

        def emit_acc(kb, eb):
            for h in range(2):
                nc.tensor.matmul(
                    acc[:, 0:512],
                    lhsT=masks[:, h * P : (h + 1) * P],
                    rhs=eb[:, h * 512 : (h + 1) * 512],
                    start=(kb == 0 and h == 0),
                    stop=(kb == NKB - 1 and h == 1),
                    skip_group_check=True,
                )

        def super_step(b, w, kb):
            ps_a = strm.tile([P, AW], f32, tag="s")
            ps_d = strm.tile([P, AW], f32, tag="s")
            for h in range(2):
                nc.tensor.matmul(
                    ps_d[:, h * 512 : (h + 1) * 512],
                    lhsT=xt2b_sb[C:P, kb * P : (kb + 1) * P],
                    rhs=xbt_sb[C:P, h * 512 : (h + 1) * 512],
                    start=True,
                    stop=True,
                )
                nc.tensor.matmul(
                    ps_a[:, h * 512 : (h + 1) * 512],
                    lhsT=xbt_sb[0:C, b * P : (b + 1) * P],
                    rhs=xt2a_sb[0:C, w * AW + h * 512 : w * AW + (h + 1) * 512],
                    start=True,
                    stop=True,
                )
            if w == 0:
                # kill self-similarity (local col b*128+p) pre-exp
                nc.tensor.matmul(
                    ps_a[:, b * P : (b + 1) * P],
                    lhsT=idn_sb[:],
                    rhs=eye_sb[:],
                    start=False,
                    stop=True,
                    skip_group_check=True,
                )
            if pending_acc:
                emit_acc(*pending_acc.pop())
            scr = scrp.tile([P, AW], bf16, tag="scr")
            idx = b * NACH + w
            nc.scalar.activation(
                scr[:], ps_a[:], AF.Exp, bias=nshift[:],
                accum_out=esum[:, idx : idx + 1],
            )
            et = ep.tile([P, AW], i16, tag="E")
            nc.vector.tensor_scalar(
                out=et[:], in0=ps_d[:], scalar1=0.0, scalar2=AEXP,
                op0=ALU.max, op1=ALU.mult,
            )
            pending_acc.append((kb, et[:].bitcast(bf16)))
            last_eb[0] = et[:].bitcast(bf16)

        last_eb = [None]
        a_list = [(b, w) for b in range(NBLK) for w in range(NACH)]
        for step in range(NKB):
            b, w = a_list[step]
            super_step(b, w, step)
        while pending_acc:
            emit_acc(*pending_acc.pop())

        # ---- G = onehot @ S (needs the AllReduced s_sb) ----
        # (the readback DMA is emitted HERE, after the whole stream, so its
        # wait on the collective never blocks stream bookkeeping queued
        # behind it on the gpsimd engine)
        nc.gpsimd.dma_start(s_sb[:], s_out[:])  # casts f32 -> bf16
        g_ps = auxp.tile([P, 512], f32, tag="aux")
        # zero-init anchor reading the last E tile: forces the scheduler to
        # order this (collective-gated) section after the whole exp stream
        nc.tensor.matmul(
            g_ps[:, 0:512],
            lhsT=zmask[:],
            rhs=last_eb[0][:, 0:512],
            start=True,
            stop=False,
            skip_group_check=True,
        )
        for b in range(NBLK):
            nc.tensor.matmul(
                g_ps[:, b * C : (b + 1) * C],
                lhsT=ohbt_sb[:, b * P : (b + 1) * P],
                rhs=s_sb[:, 0:C],
                start=False,
                stop=(b == NBLK - 1),
                skip_group_check=True,
            )
        t2 = p3s.tile([P, NBLK * C], f32, tag="p3")
        t2_blk = t2[:].rearrange("p (b c) -> p b c", c=C)
        nc.vector.tensor_mul(t2_blk, xaug_blk, g_ps[:].rearrange("p (b c) -> p b c", c=C))
        nc.vector.reduce_sum(poss[:], t2_blk, axis=AX.X)
        cnt_ps = auxp.tile([P, 512], f32, tag="aux")
        for b in range(NBLK):
            nc.tensor.matmul(
                cnt_ps[:, b : b + 1],
                lhsT=ohbt_sb[:, b * P : (b + 1) * P],
                rhs=s_sb[:, C : C + 1],
                start=True,
                stop=True,
            )
        nc.vector.tensor_copy(cnt[:], cnt_ps[:, 0:NBLK])

        # ---- tail: combine rowsums ----
        # ACT-side per-block rowsums
        nc.vector.reduce_sum(
            se[:], esum[:].rearrange("p (b w) -> p b w", w=NACH), axis=AX.X
        )
        # DVE-side: drain accum, transpose to row-major, gather
        nc.vector.tensor_copy(acc_sb[:], acc[:])
        tps = auxp.tile([P, 512], bf16, tag="aux")
        for w in range(4):
            nc.tensor.transpose(
                tps[:, w * P : (w + 1) * P],
                acc_sb[:, w * P : (w + 1) * P],
                idn_sb[:],
            )
        # seD[p, b] with b = h*4 + w at tps[p, w*128 + h*64]
        tq = tps[:].rearrange("p (w q r) -> p w q r", w=4, q=2)
        seD_v = seD[:].rearrange("p (h w o) -> p w h o", w=4, o=1)
        nc.vector.tensor_copy(seD_v, tq[:, :, 0:2, 0:1])

        # se_tot = se + KDV * seD
        se_tot = stats.tile([P, NBLK], f32)
        nc.vector.scalar_tensor_tensor(
            out=se_tot[:], in0=seD[:], scalar=KDV, in1=se[:],
            op0=ALU.mult, op1=ALU.add,
        )

        # robust ln via exponent/mantissa split; ln(mantissa) by a DVE
        # cubic (keeps the Ln ACT-table load off the kernel entirely)
        LC = np.polyfit(
            np.cos(np.linspace(0, np.pi, 64)) * 0.5 + 1.5,
            np.log(np.cos(np.linspace(0, np.pi, 64)) * 0.5 + 1.5),
            3,
        )  # c3..c0 for ln(m), m in [1,2]
        fin = stats

        def emit_ln(dst, src, nm):
            sec = fin.tile([P, NBLK], f32, name=f"{nm}_sec")
            nc.vector.tensor_scalar_max(sec[:], src, 1e-37)
            bits = sec[:].bitcast(i32)
            exi = fin.tile([P, NBLK], i32, name=f"{nm}_exi")
            nc.vector.tensor_scalar(
                out=exi[:], in0=bits, scalar1=23, scalar2=None,
                op0=ALU.arith_shift_right,
            )
            exf = fin.tile([P, NBLK], f32, name=f"{nm}_exf")
            nc.vector.tensor_copy(exf[:], exi[:])
            mbits = fin.tile([P, NBLK], i32, name=f"{nm}_mb")
            nc.vector.tensor_scalar(
                out=mbits[:], in0=bits, scalar1=0x007FFFFF, scalar2=0x3F800000,
                op0=ALU.bitwise_and, op1=ALU.bitwise_or,
            )
            m = mbits[:].bitcast(f32)
            t1 = fin.tile([P, NBLK], f32, name=f"{nm}_t1")
            nc.vector.tensor_scalar(
                out=t1[:], in0=m, scalar1=float(LC[0]), scalar2=float(LC[1]),
                op0=ALU.mult, op1=ALU.add,
            )
            t2 = fin.tile([P, NBLK], f32, name=f"{nm}_t2")
            nc.vector.tensor_mul(t2[:], t1[:], m)
            t3 = fin.tile([P, NBLK], f32, name=f"{nm}_t3")
            nc.vector.tensor_scalar(
                out=t3[:], in0=t2[:], scalar1=float(LC[2]), scalar2=None,
                op0=ALU.add,
            )
            t4 = fin.tile([P, NBLK], f32, name=f"{nm}_t4")
            nc.vector.tensor_mul(t4[:], t3[:], m)
            # dst = t4 + c0 + (exf - 127)*ln2
            t5 = fin.tile([P, NBLK], f32, name=f"{nm}_t5")
            nc.vector.tensor_scalar(
                out=t5[:], in0=exf[:], scalar1=-127.0,
                scalar2=float(np.log(2.0)), op0=ALU.add, op1=ALU.mult,
            )
            t6 = fin.tile([P, NBLK], f32, name=f"{nm}_t6")
            nc.vector.tensor_scalar(
                out=t6[:], in0=t4[:], scalar1=float(LC[3]), scalar2=None,
                op0=ALU.add,
            )
            nc.vector.tensor_add(dst, t6[:], t5[:])

        lnse = fin.tile([P, NBLK], f32)
        emit_ln(lnse[:], se_tot[:], "lnse")

        nposc = fin.tile([P, NBLK], f32)
        nc.vector.tensor_scalar(
            out=nposc[:], in0=cnt[:], scalar1=-1.0, scalar2=1.0,
            op0=ALU.add, op1=ALU.max,
        )
        mask = fin.tile([P, NBLK], f32)
        nc.vector.tensor_scalar(
            out=mask[:], in0=cnt[:], scalar1=-1.0, scalar2=1.0,
            op0=ALU.add, op1=ALU.min,
        )
        rc = fin.tile([P, NBLK], f32)
        nc.vector.reciprocal(rc[:], nposc[:])

        pd = fin.tile([P, NBLK], f32)
        nc.vector.tensor_sub(pd[:], poss[:], nrm[:])
        pt = fin.tile([P, NBLK], f32)
        nc.vector.scalar_tensor_tensor(
            out=pt[:], in0=pd[:], scalar=1.0 / TAU, in1=rc[:],
            op0=ALU.mult, op1=ALU.mult,
        )
        peri = fin.tile([P, NBLK], f32)
        nc.vector.scalar_tensor_tensor(
            out=peri[:], in0=lnse[:], scalar=SHIFT, in1=pt[:],
            op0=ALU.add, op1=ALU.subtract,
        )
        perim = fin.tile([P, NBLK], f32)
        nc.vector.tensor_mul(perim[:], peri[:], mask[:])

        lnce = fin.tile([P, NBLK], f32)
        emit_ln(lnce[:], cesum[:], "lnce")
        cec = fin.tile([P, NBLK], f32)
        nc.vector.tensor_sub(cec[:], lnce[:], lgt[:])

        nc.vector.reduce_sum(res[:, 0:1], perim[:], axis=AX.X)
        nc.vector.reduce_sum(res[:, 1:2], cec[:], axis=AX.X)
        nc.sync.dma_start(out_d.ap(), res[:])

    with tile.TileContext(nc) as tc, ExitStack() as ctx:
        emit(tc, ctx)

    nc.compile()
    return nc


def _get_nc(**kw):
    key = repr(sorted(kw.items()))
    if key not in _CACHE:
        _CACHE[key] = _build(**kw)
    return _CACHE[key]


def _make_in_maps(X, y):
    import ml_dtypes

    bf = ml_dtypes.bfloat16
    X = np.ascontiguousarray(np.asarray(X, dtype=np.float32))
    y = np.asarray(y).astype(np.int64).ravel()
    assert X.shape == (N, C) and y.shape == (N,)

    oh = (y[:, None] == np.arange(C)[None, :]).astype(np.float32)
    xt2 = np.ascontiguousarray((X.T / np.float32(TAU)).astype(bf))
    eyeneg = (np.eye(P) * -1e4).astype(bf)
    ident = np.eye(P).astype(bf)

    in_maps = []
    for r in range(NCORES):
        rows = slice(r * RPC, (r + 1) * RPC)
        xb = X[rows]
        xt2r = np.roll(xt2, -r * RPC, axis=1)
        xaug = np.concatenate([xb, np.ones((RPC, 1), np.float32)], axis=1)
        # per-block layouts [p, b*K+c], contiguous for straight DMA
        xaug_pb = np.ascontiguousarray(
            xaug.reshape(NBLK, P, C + 1).transpose(1, 0, 2).reshape(P, -1)
        ).astype(bf)
        ohb_pb = np.ascontiguousarray(
            oh[rows].reshape(NBLK, P, C).transpose(1, 0, 2).reshape(P, -1)
        ).astype(bf)
        in_maps.append(
            {
                "xt2a": np.ascontiguousarray(xt2r[:, :ACOLS]),
                "xt2b": np.ascontiguousarray(xt2r[:, ACOLS:]),
                "xbt": np.ascontiguousarray(xb.T.astype(bf)),
                "xaug": xaug_pb,
                "ohb": ohb_pb,
                "ohbt": np.ascontiguousarray(oh[rows].T.astype(bf)),
                "eyeneg": eyeneg,
                "ident": ident,
            }
        )
    return in_maps


def run(input, target, trace=False, **build_kw):
    """Run the device kernel; returns (loss_scalar, BassKernelResults)."""
    from concourse.bass_utils import run_bass_kernel_spmd

    nc = _get_nc(**build_kw)
    in_maps = _make_in_maps(input, target)
    res = run_bass_kernel_spmd(
        nc, in_maps, core_ids=list(range(NCORES)), trace=trace
    )
    sc = 0.0
    ce = 0.0
    for core_out in res.results:
        o = core_out["out"].astype(np.float64)
        sc += o[:, 0].sum()
        ce += o[:, 1].sum()
    loss = (1.0 - LMBD) * (ce / N) + LMBD * sc
    return np.array(loss, dtype=np.float32), res


def kernel(input, target):
    loss, _ = run(input, target, trace=False)
    return loss


# revision 21
# speedup vs baseline: 1.7395x; 1.3905x over previous
"""Fused CE + supervised-contrastive loss on 8 Trainium2 NeuronCores.

Math (reference semantics):
  ce   = -mean_i log_softmax(input)[i, y_i]
  sim  = (X @ X.T) / tau, diag excluded
  lse_i = logsumexp_{k!=i} sim[i,k]
  possum_i = (x_i . S_{y_i} - ||x_i||^2)/tau, S_c = sum_{k: y_k=c} x_k
  per_i = lse_i - possum_i/n_pos_i  (0 if n_pos_i == 0)
  loss = (1-lmbd)*ce + lmbd * sum_i per_i

The ONLY O(N^2) term is the row-wise sum of exp(sim); the device computes
exactly that (8.4M exps/core), split across two engines working in
parallel on a shared 3-slot PSUM rotation:

  * ACT path (local cols 0..ACOLS): row-major sim chunks [128 rows, 1024].
    ScalarE exp with bias=-SHIFT and fused accum_out row-sums (esum).
    The diagonal (always in local cols [0,1024) thanks to the per-core xt2
    rotation) is killed pre-exp by a diag(-1e4) accumulate-matmul.
  * DVE path (cols ACOLS..8192): TRANSPOSED sim chunks [128 cols, 1024
    rows] (lhsT = xt2 column block, rhs = xbt). VectorE computes a
    one-instruction Schraudolph fast-exp: bits16 = int16(max(sim,0) *
    128/ln2), bitcast to bf16 == e^(sim - 127*ln2) * rho, rho in
    [1, 1.086] (measured mean 1.0410 on HW, folded into KDV). The clamp is
    mandatory: the int16 convert WRAPS on negative; clamped terms
    contribute exactly +0.0 (true value < e^-88, negligible).
    The PE row-sums the bf16 tiles with masked-ones lhsT matmuls (mask
    columns 0-63 select out-rows 0-63 for row-half 0, 64-127 for half 1)
    so BOTH halves accumulate into one standard [128, 512] PSUM bank at
    tile_position (0,0) -- col-tiled M=1 outputs proved broken. A 4x
    PE-transpose at the tail converts rows-in-free to row-major [128, 8].

  PE lanes: ACT fills use lo operand copies (array rows 0-63), DVE fills
  hi copies (rows 64-127), emitted interleaved for row-group overlap.
  Each chunk's reduce matmuls are deferred one super-step so the in-order
  PE queue never waits on DVE's fast-exp.

Everything O(N*C) -- class sums, positive-pair sums, counts, row norms,
logit gather, the CE term, and the final combine -- runs on the host in
float64 alongside the input prep (rotations/onehots/casts). This removes
the AllReduce whose enqueue-barrier (~47us of cross-core skew) + ~14us
ncfw latency gated the previous design's tail.

Outputs per core: [128, 8] se_act (shift-100 domain) and [128, 8] seD
(fast-exp domain); host computes lse = ln(se + KDV*seD) + SHIFT.
"""

import math

import numpy as np

N, C = 8192, 64
NCORES = 8
RPC = N // NCORES          # rows per core (1024)
P = 128                    # partitions per row-block
NBLK = RPC // P            # 8 row blocks per core
TAU = 0.5
LMBD = 0.5
SHIFT = 100.0              # ACT-path logsumexp shift
ACOLS = 4096               # ACT row-major columns per core
DCOLS = N - ACOLS          # DVE transposed columns per core
AW = 1024                  # chunk width
NACH = ACOLS // AW         # ACT chunks per block (4)
NKB = DCOLS // P           # DVE column blocks (32)
AEXP = 128.0 / math.log(2.0)          # fast-exp scale (184.6646)
RHO = 1.0410                          # measured mean Schraudolph ratio (HW)
KDV = math.exp(127.0 * math.log(2.0) - SHIFT) / RHO

_CACHE = {}


def _build():
    from contextlib import ExitStack

    import concourse.bass as bass
    import concourse.tile as tile
    from concourse import bacc, mybir

    f32 = mybir.dt.float32
    i16 = mybir.dt.int16
    bf16 = mybir.dt.bfloat16
    AF = mybir.ActivationFunctionType
    ALU = mybir.AluOpType
    AX = mybir.AxisListType

    nc = bacc.Bacc(
        "TRN2",
        target_bir_lowering=False,
        debug=False,
        num_devices=NCORES,
    )

    xt2a_d = nc.dram_tensor("xt2a", [C, ACOLS], bf16, kind="ExternalInput")
    xt2b_d = nc.dram_tensor("xt2b", [C, DCOLS], bf16, kind="ExternalInput")
    xbt_d = nc.dram_tensor("xbt", [C, RPC], bf16, kind="ExternalInput")
    eye_d = nc.dram_tensor("eyeneg", [P, P], bf16, kind="ExternalInput")
    idn_d = nc.dram_tensor("ident", [P, P], bf16, kind="ExternalInput")
    out_d = nc.dram_tensor("out", [P, 16], f32, kind="ExternalOutput")

    def emit(tc, ctx):
        const = ctx.enter_context(tc.tile_pool(name="const", bufs=1))
        strm = ctx.enter_context(tc.tile_pool(name="strm", bufs=3, space="PSUM"))
        accp = ctx.enter_context(tc.tile_pool(name="accp", bufs=1, space="PSUM"))
        auxp = ctx.enter_context(tc.tile_pool(name="auxp", bufs=1, space="PSUM"))
        scrp = ctx.enter_context(tc.tile_pool(name="scrp", bufs=2))
        ep = ctx.enter_context(tc.tile_pool(name="ep", bufs=3))
        stats = ctx.enter_context(tc.tile_pool(name="stats", bufs=1))

        # ---- input DMAs on two queues; first-chunk operands lead ----
        xbt_sb = const.tile([P, RPC], bf16)
        nc.sync.dma_start(xbt_sb[0:C, :], xbt_d.ap())
        nc.sync.dma_start(xbt_sb[C:P, :], xbt_d.ap())
        xt2b_sb = const.tile([P, DCOLS], bf16)
        nc.gpsimd.dma_start(xt2b_sb[C:P, 0:AW], xt2b_d.ap()[:, 0:AW])
        xt2a_sb = const.tile([P, ACOLS], bf16)
        nc.sync.dma_start(xt2a_sb[0:C, 0:AW], xt2a_d.ap()[:, 0:AW])
        eye_sb = const.tile([P, P], bf16)
        nc.sync.dma_start(eye_sb[:], eye_d.ap())
        idn_sb = const.tile([P, P], bf16)
        nc.sync.dma_start(idn_sb[:], idn_d.ap())
        nc.sync.dma_start(xt2a_sb[0:C, AW:ACOLS], xt2a_d.ap()[:, AW:ACOLS])
        nc.gpsimd.dma_start(xt2b_sb[C:P, AW:DCOLS], xt2b_d.ap()[:, AW:DCOLS])

        # ---- persistent small tiles ----
        nshift = stats.tile([P, 1], f32)
        nc.vector.memset(nshift[:], -SHIFT)
        # masked-ones lhsT: mh0 -> out rows 0-63 (row-half 0), mh1 -> 64-127
        masks = stats.tile([P, 2 * P], bf16)
        nc.vector.memset(masks[:, 0:C], 1.0)
        nc.vector.memset(masks[:, C:P], 0.0)
        nc.vector.memset(masks[:, P : P + C], 0.0)
        nc.vector.memset(masks[:, P + C : 2 * P], 1.0)
        esum = stats.tile([P, NBLK * NACH], f32)
        acc_sb = stats.tile([P, 512], bf16)
        res = stats.tile([P, 16], f32)

        # rowsum accumulator: rows 0-63 = row-half 0 (redundant copies),
        # rows 64-127 = half 1; free = row-within-half
        acc = accp.tile([P, 512], f32)

        # ---- main interleaved exp stream ----
        pending_acc = []

        def emit_acc(kb, eb):
            for h in range(2):
                nc.tensor.matmul(
                    acc[:, 0:512],
                    lhsT=masks[:, h * P : (h + 1) * P],
                    rhs=eb[:, h * 512 : (h + 1) * 512],
                    start=(kb == 0 and h == 0),
                    stop=(kb == NKB - 1 and h == 1),
                    skip_group_check=True,
                )

        def super_step(b, w, kb):
            ps_a = strm.tile([P, AW], f32, tag="s")
            ps_d = strm.tile([P, AW], f32, tag="s")
            for h in range(2):
                nc.tensor.matmul(
                    ps_d[:, h * 512 : (h + 1) * 512],
                    lhsT=xt2b_sb[C:P, kb * P : (kb + 1) * P],
                    rhs=xbt_sb[C:P, h * 512 : (h + 1) * 512],
                    start=True,
                    stop=True,
                )
                nc.tensor.matmul(
                    ps_a[:, h * 512 : (h + 1) * 512],
                    lhsT=xbt_sb[0:C, b * P : (b + 1) * P],
                    rhs=xt2a_sb[0:C, w * AW + h * 512 : w * AW + (h + 1) * 512],
                    start=True,
                    stop=True,
                )
            if w == 0:
                # kill self-similarity (local col b*128+p) pre-exp
                nc.tensor.matmul(
                    ps_a[:, b * P : (b + 1) * P],
                    lhsT=idn_sb[:],
                    rhs=eye_sb[:],
                    start=False,
                    stop=True,
                    skip_group_check=True,
                )
            if pending_acc:
                emit_acc(*pending_acc.pop())
            scr = scrp.tile([P, AW], bf16, tag="scr")
            idx = b * NACH + w
            nc.scalar.activation(
                scr[:], ps_a[:], AF.Exp, bias=nshift[:],
                accum_out=esum[:, idx : idx + 1],
            )
            et = ep.tile([P, AW], i16, tag="E")
            nc.vector.tensor_scalar(
                out=et[:], in0=ps_d[:], scalar1=0.0, scalar2=AEXP,
                op0=ALU.max, op1=ALU.mult,
            )
            pending_acc.append((kb, et[:].bitcast(bf16)))

        a_list = [(b, w) for b in range(NBLK) for w in range(NACH)]
        for step in range(NKB):
            b, w = a_list[step]
            super_step(b, w, step)
        while pending_acc:
            emit_acc(*pending_acc.pop())

        # ---- tail: per-block rowsums out ----
        nc.vector.reduce_sum(
            res[:, 0:NBLK],
            esum[:].rearrange("p (b w) -> p b w", w=NACH),
            axis=AX.X,
        )
        nc.vector.tensor_copy(acc_sb[:], acc[:])
        tps = auxp.tile([P, 512], bf16, tag="aux")
        for w in range(4):
            nc.tensor.transpose(
                tps[:, w * P : (w + 1) * P],
                acc_sb[:, w * P : (w + 1) * P],
                idn_sb[:],
            )
        # seD[p, b] with b = h*4 + w sits at tps[p, w*128 + h*64]
        tq = tps[:].rearrange("p (w q r) -> p w q r", w=4, q=2)
        seD_v = res[:, NBLK : 2 * NBLK].rearrange("p (h w o) -> p w h o", w=4, o=1)
        nc.vector.tensor_copy(seD_v, tq[:, :, 0:2, 0:1])
        nc.sync.dma_start(out_d.ap(), res[:])

    with tile.TileContext(nc) as tc, ExitStack() as ctx:
        emit(tc, ctx)

    nc.compile()
    return nc


def _get_nc(**kw):
    key = repr(sorted(kw.items()))
    if key not in _CACHE:
        _CACHE[key] = _build(**kw)
    return _CACHE[key]


def _make_in_maps(X, y):
    import ml_dtypes

    bf = ml_dtypes.bfloat16
    X = np.ascontiguousarray(np.asarray(X, dtype=np.float32))
    assert X.shape == (N, C)

    xt2 = np.ascontiguousarray((X.T / np.float32(TAU)).astype(bf))
    eyeneg = (np.eye(P) * -1e4).astype(bf)
    ident = np.eye(P).astype(bf)

    in_maps = []
    for r in range(NCORES):
        rows = slice(r * RPC, (r + 1) * RPC)
        xb = X[rows]
        xt2r = np.roll(xt2, -r * RPC, axis=1)
        in_maps.append(
            {
                "xt2a": np.ascontiguousarray(xt2r[:, :ACOLS]),
                "xt2b": np.ascontiguousarray(xt2r[:, ACOLS:]),
                "xbt": np.ascontiguousarray(xb.T.astype(bf)),
                "eyeneg": eyeneg,
                "ident": ident,
            }
        )
    return in_maps


def run(input, target, trace=False, **build_kw):
    """Run the device kernel; returns (loss_scalar, BassKernelResults)."""
    from concourse.bass_utils import run_bass_kernel_spmd

    nc = _get_nc(**build_kw)
    X = np.ascontiguousarray(np.asarray(input, dtype=np.float32))
    y = np.asarray(target).astype(np.int64).ravel()
    in_maps = _make_in_maps(X, y)
    res = run_bass_kernel_spmd(
        nc, in_maps, core_ids=list(range(NCORES)), trace=trace
    )

    # device gave per-row exp sums; the O(N*C) remainder runs here in f64
    se = np.empty(N)
    for r, core_out in enumerate(res.results):
        o = core_out["out"].astype(np.float64)  # [128, 16]
        se_act = o[:, 0:NBLK]    # [128, 8], row b*128+p -> [p, b]
        se_dve = o[:, NBLK:16]
        tot = se_act + KDV * se_dve              # shift-SHIFT domain
        se[r * RPC:(r + 1) * RPC] = np.maximum(tot.T.ravel(), 1e-300)

    Xd = X.astype(np.float64)
    lse = np.log(se) + SHIFT                     # [N]
    S = np.zeros((C, C))
    np.add.at(S, y, Xd)
    counts = np.bincount(y, minlength=C).astype(np.float64)
    n_pos = counts[y] - 1.0
    possum = ((Xd * S[y]).sum(axis=1) - (Xd * Xd).sum(axis=1)) / TAU
    per_i = np.where(n_pos > 0, lse - possum / np.maximum(n_pos, 1.0), 0.0)
    sc = per_i.sum()

    m = Xd.max(axis=1)
    ce_lse = np.log(np.exp(Xd - m[:, None]).sum(axis=1)) + m
    ce = (ce_lse - Xd[np.arange(N), y]).mean()

    loss = (1.0 - LMBD) * ce + LMBD * sc
    return np.array(loss, dtype=np.float32), res


def kernel(input, target):
    loss, _ = run(input, target, trace=False)
    return loss


# revision 22
# speedup vs baseline: 1.7776x; 1.0219x over previous
"""Fused CE + supervised-contrastive loss on 8 Trainium2 NeuronCores.

Math (reference semantics):
  ce   = -mean_i log_softmax(input)[i, y_i]
  sim  = (X @ X.T) / tau, diag excluded
  lse_i = logsumexp_{k!=i} sim[i,k]
  possum_i = (x_i . S_{y_i} - ||x_i||^2)/tau, S_c = sum_{k: y_k=c} x_k
  per_i = lse_i - possum_i/n_pos_i  (0 if n_pos_i == 0)
  loss = (1-lmbd)*ce + lmbd * sum_i per_i

The ONLY O(N^2) term is the row-wise sum of exp(sim); the device computes
exactly that (8.4M exps/core), split across two engines working in
parallel on a shared 3-slot PSUM rotation:

  * ACT path (local cols 0..ACOLS): row-major sim chunks [128 rows, 1024].
    ScalarE exp with bias=-SHIFT and fused accum_out row-sums (esum).
    The diagonal (always in local cols [0,1024) thanks to the per-core xt2
    rotation) is killed pre-exp by a diag(-1e4) accumulate-matmul.
  * DVE path (cols ACOLS..8192): TRANSPOSED sim chunks [128 cols, 1024
    rows] (lhsT = xt2 column block, rhs = xbt). VectorE computes a
    one-instruction Schraudolph fast-exp: bits16 = int16(max(sim,0) *
    128/ln2), bitcast to bf16 == e^(sim - 127*ln2) * rho, rho in
    [1, 1.086] (measured mean 1.0410 on HW, folded into KDV). The clamp is
    mandatory: the int16 convert WRAPS on negative; clamped terms
    contribute exactly +0.0 (true value < e^-88, negligible).
    The PE row-sums the bf16 tiles with masked-ones lhsT matmuls (mask
    columns 0-63 select out-rows 0-63 for row-half 0, 64-127 for half 1)
    so BOTH halves accumulate into one standard [128, 512] PSUM bank at
    tile_position (0,0) -- col-tiled M=1 outputs proved broken. A 4x
    PE-transpose at the tail converts rows-in-free to row-major [128, 8].

  PE lanes: ACT fills use lo operand copies (array rows 0-63), DVE fills
  hi copies (rows 64-127), emitted interleaved for row-group overlap.
  Each chunk's reduce matmuls are deferred one super-step so the in-order
  PE queue never waits on DVE's fast-exp.

Everything O(N*C) -- class sums, positive-pair sums, counts, row norms,
logit gather, the CE term, and the final combine -- runs on the host in
float64 alongside the input prep (rotations/onehots/casts). This removes
the AllReduce whose enqueue-barrier (~47us of cross-core skew) + ~14us
ncfw latency gated the previous design's tail.

Outputs per core: [128, 8] se_act (shift-100 domain) and [128, 8] seD
(fast-exp domain); host computes lse = ln(se + KDV*seD) + SHIFT.
"""

import math

import numpy as np

N, C = 8192, 64
NCORES = 8
RPC = N // NCORES          # rows per core (1024)
P = 128                    # partitions per row-block
NBLK = RPC // P            # 8 row blocks per core
TAU = 0.5
LMBD = 0.5
SHIFT = 100.0              # ACT-path logsumexp shift
ACOLS = 4096               # ACT row-major columns per core
DCOLS = N - ACOLS          # DVE transposed columns per core
AW = 1024                  # chunk width
NACH = ACOLS // AW         # ACT chunks per block (4)
NKB = DCOLS // P           # DVE column blocks (32)
AEXP = 128.0 / math.log(2.0)          # fast-exp scale (184.6646)
RHO = 1.0410                          # measured mean Schraudolph ratio (HW)
KDV = math.exp(127.0 * math.log(2.0) - SHIFT) / RHO

_CACHE = {}


def _build():
    from contextlib import ExitStack

    import concourse.bass as bass
    import concourse.tile as tile
    from concourse import bacc, mybir

    f32 = mybir.dt.float32
    i16 = mybir.dt.int16
    bf16 = mybir.dt.bfloat16
    AF = mybir.ActivationFunctionType
    ALU = mybir.AluOpType
    AX = mybir.AxisListType

    nc = bacc.Bacc(
        "TRN2",
        target_bir_lowering=False,
        debug=False,
        num_devices=NCORES,
    )

    xt2a_d = nc.dram_tensor("xt2a", [C, ACOLS], bf16, kind="ExternalInput")
    xt2b_d = nc.dram_tensor("xt2b", [C, DCOLS], bf16, kind="ExternalInput")
    xbt_d = nc.dram_tensor("xbt", [C, RPC], bf16, kind="ExternalInput")
    eye_d = nc.dram_tensor("eyeneg", [P, P], bf16, kind="ExternalInput")
    idn_d = nc.dram_tensor("ident", [P, P], bf16, kind="ExternalInput")
    out_d = nc.dram_tensor("out", [P, 16], f32, kind="ExternalOutput")

    def emit(tc, ctx):
        const = ctx.enter_context(tc.tile_pool(name="const", bufs=1))
        strm = ctx.enter_context(tc.tile_pool(name="strm", bufs=3, space="PSUM"))
        accp = ctx.enter_context(tc.tile_pool(name="accp", bufs=1, space="PSUM"))
        auxp = ctx.enter_context(tc.tile_pool(name="auxp", bufs=1, space="PSUM"))
        scrp = ctx.enter_context(tc.tile_pool(name="scrp", bufs=2))
        ep = ctx.enter_context(tc.tile_pool(name="ep", bufs=3))
        stats = ctx.enter_context(tc.tile_pool(name="stats", bufs=1))

        # ---- input DMAs on two queues; first-chunk operands lead ----
        xbt_sb = const.tile([P, RPC], bf16)
        nc.sync.dma_start(xbt_sb[0:C, :], xbt_d.ap())
        nc.sync.dma_start(xbt_sb[C:P, :], xbt_d.ap())
        xt2b_sb = const.tile([P, DCOLS], bf16)
        nc.sync.dma_start(xt2b_sb[C:P, 0:AW], xt2b_d.ap()[:, 0:AW])
        xt2a_sb = const.tile([P, ACOLS], bf16)
        nc.sync.dma_start(xt2a_sb[0:C, 0:AW], xt2a_d.ap()[:, 0:AW])
        eye_sb = const.tile([P, P], bf16)
        nc.sync.dma_start(eye_sb[:], eye_d.ap())
        idn_sb = const.tile([P, P], bf16)
        nc.sync.dma_start(idn_sb[:], idn_d.ap())
        nc.sync.dma_start(xt2a_sb[0:C, AW:ACOLS], xt2a_d.ap()[:, AW:ACOLS])
        nc.sync.dma_start(xt2b_sb[C:P, AW:DCOLS], xt2b_d.ap()[:, AW:DCOLS])

        # ---- persistent small tiles ----
        nshift = stats.tile([P, 1], f32)
        nc.vector.memset(nshift[:], -SHIFT)
        # masked-ones lhsT: mh0 -> out rows 0-63 (row-half 0), mh1 -> 64-127
        masks = stats.tile([P, 2 * P], bf16)
        nc.vector.memset(masks[:, 0:C], 1.0)
        nc.vector.memset(masks[:, C:P], 0.0)
        nc.vector.memset(masks[:, P : P + C], 0.0)
        nc.vector.memset(masks[:, P + C : 2 * P], 1.0)
        esum = stats.tile([P, NBLK * NACH], f32)
        acc_sb = stats.tile([P, 512], bf16)
        res = stats.tile([P, 16], f32)

        # rowsum accumulator: rows 0-63 = row-half 0 (redundant copies),
        # rows 64-127 = half 1; free = row-within-half
        acc = accp.tile([P, 512], f32)

        # ---- main interleaved exp stream ----
        pending_acc = []

        def emit_acc(kb, eb):
            for h in range(2):
                nc.tensor.matmul(
                    acc[:, 0:512],
                    lhsT=masks[:, h * P : (h + 1) * P],
                    rhs=eb[:, h * 512 : (h + 1) * 512],
                    start=(kb == 0 and h == 0),
                    stop=(kb == NKB - 1 and h == 1),
                    skip_group_check=True,
                )

        def super_step(b, w, kb):
            ps_a = strm.tile([P, AW], f32, tag="s")
            ps_d = strm.tile([P, AW], f32, tag="s")
            for h in range(2):
                nc.tensor.matmul(
                    ps_d[:, h * 512 : (h + 1) * 512],
                    lhsT=xt2b_sb[C:P, kb * P : (kb + 1) * P],
                    rhs=xbt_sb[C:P, h * 512 : (h + 1) * 512],
                    start=True,
                    stop=True,
                )
                nc.tensor.matmul(
                    ps_a[:, h * 512 : (h + 1) * 512],
                    lhsT=xbt_sb[0:C, b * P : (b + 1) * P],
                    rhs=xt2a_sb[0:C, w * AW + h * 512 : w * AW + (h + 1) * 512],
                    start=True,
                    stop=True,
                )
            if w == 0:
                # kill self-similarity (local col b*128+p) pre-exp
                nc.tensor.matmul(
                    ps_a[:, b * P : (b + 1) * P],
                    lhsT=idn_sb[:],
                    rhs=eye_sb[:],
                    start=False,
                    stop=True,
                    skip_group_check=True,
                )
            if pending_acc:
                emit_acc(*pending_acc.pop())
            scr = scrp.tile([P, AW], bf16, tag="scr")
            idx = b * NACH + w
            nc.scalar.activation(
                scr[:], ps_a[:], AF.Exp, bias=nshift[:],
                accum_out=esum[:, idx : idx + 1],
            )
            et = ep.tile([P, AW], i16, tag="E")
            nc.vector.tensor_scalar(
                out=et[:], in0=ps_d[:], scalar1=0.0, scalar2=AEXP,
                op0=ALU.max, op1=ALU.mult,
            )
            pending_acc.append((kb, et[:].bitcast(bf16)))

        a_list = [(b, w) for b in range(NBLK) for w in range(NACH)]
        for step in range(NKB):
            b, w = a_list[step]
            super_step(b, w, step)
        while pending_acc:
            emit_acc(*pending_acc.pop())

        # ---- tail: per-block rowsums out ----
        nc.vector.reduce_sum(
            res[:, 0:NBLK],
            esum[:].rearrange("p (b w) -> p b w", w=NACH),
            axis=AX.X,
        )
        nc.vector.tensor_copy(acc_sb[:], acc[:])
        tps = auxp.tile([P, 512], bf16, tag="aux")
        for w in range(4):
            nc.tensor.transpose(
                tps[:, w * P : (w + 1) * P],
                acc_sb[:, w * P : (w + 1) * P],
                idn_sb[:],
            )
        # seD[p, b] with b = h*4 + w sits at tps[p, w*128 + h*64]
        tq = tps[:].rearrange("p (w q r) -> p w q r", w=4, q=2)
        seD_v = res[:, NBLK : 2 * NBLK].rearrange("p (h w o) -> p w h o", w=4, o=1)
        nc.vector.tensor_copy(seD_v, tq[:, :, 0:2, 0:1])
        nc.sync.dma_start(out_d.ap(), res[:])

    with tile.TileContext(nc) as tc, ExitStack() as ctx:
        emit(tc, ctx)

    nc.compile()
    return nc


def _get_nc(**kw):
    key = repr(sorted(kw.items()))
    if key not in _CACHE:
        _CACHE[key] = _build(**kw)
    return _CACHE[key]


def _make_in_maps(X, y):
    import ml_dtypes

    bf = ml_dtypes.bfloat16
    X = np.ascontiguousarray(np.asarray(X, dtype=np.float32))
    assert X.shape == (N, C)

    xt2 = np.ascontiguousarray((X.T / np.float32(TAU)).astype(bf))
    eyeneg = (np.eye(P) * -1e4).astype(bf)
    ident = np.eye(P).astype(bf)

    in_maps = []
    for r in range(NCORES):
        rows = slice(r * RPC, (r + 1) * RPC)
        xb = X[rows]
        xt2r = np.roll(xt2, -r * RPC, axis=1)
        in_maps.append(
            {
                "xt2a": np.ascontiguousarray(xt2r[:, :ACOLS]),
                "xt2b": np.ascontiguousarray(xt2r[:, ACOLS:]),
                "xbt": np.ascontiguousarray(xb.T.astype(bf)),
                "eyeneg": eyeneg,
                "ident": ident,
            }
        )
    return in_maps


def run(input, target, trace=False, **build_kw):
    """Run the device kernel; returns (loss_scalar, BassKernelResults)."""
    from concourse.bass_utils import run_bass_kernel_spmd

    nc = _get_nc(**build_kw)
    X = np.ascontiguousarray(np.asarray(input, dtype=np.float32))
    y = np.asarray(target).astype(np.int64).ravel()
    in_maps = _make_in_maps(X, y)
    res = run_bass_kernel_spmd(
        nc, in_maps, core_ids=list(range(NCORES)), trace=trace
    )

    # device gave per-row exp sums; the O(N*C) remainder runs here in f64
    se = np.empty(N)
    for r, core_out in enumerate(res.results):
        o = core_out["out"].astype(np.float64)  # [128, 16]
        se_act = o[:, 0:NBLK]    # [128, 8], row b*128+p -> [p, b]
        se_dve = o[:, NBLK:16]
        tot = se_act + KDV * se_dve              # shift-SHIFT domain
        se[r * RPC:(r + 1) * RPC] = np.maximum(tot.T.ravel(), 1e-300)

    Xd = X.astype(np.float64)
    lse = np.log(se) + SHIFT                     # [N]
    S = np.zeros((C, C))
    np.add.at(S, y, Xd)
    counts = np.bincount(y, minlength=C).astype(np.float64)
    n_pos = counts[y] - 1.0
    possum = ((Xd * S[y]).sum(axis=1) - (Xd * Xd).sum(axis=1)) / TAU
    per_i = np.where(n_pos > 0, lse - possum / np.maximum(n_pos, 1.0), 0.0)
    sc = per_i.sum()

    m = Xd.max(axis=1)
    ce_lse = np.log(np.exp(Xd - m[:, None]).sum(axis=1)) + m
    ce = (ce_lse - Xd[np.arange(N), y]).mean()

    loss = (1.0 - LMBD) * ce + LMBD * sc
    return np.array(loss, dtype=np.float32), res


def kernel(input, target):
    loss, _ = run(input, target, trace=False)
    return loss


# revision 23
# speedup vs baseline: 1.8204x; 1.0241x over previous
"""Fused CE + supervised-contrastive loss on 8 Trainium2 NeuronCores.

Math (reference semantics):
  ce   = -mean_i log_softmax(input)[i, y_i]
  sim  = (X @ X.T) / tau, diag excluded
  lse_i = logsumexp_{k!=i} sim[i,k]
  possum_i = (x_i . S_{y_i} - ||x_i||^2)/tau, S_c = sum_{k: y_k=c} x_k
  per_i = lse_i - possum_i/n_pos_i  (0 if n_pos_i == 0)
  loss = (1-lmbd)*ce + lmbd * sum_i per_i

The ONLY O(N^2) term is the row-wise sum of exp(sim); the device computes
exactly that (8.4M exps/core), split across two engines working in
parallel on a shared 3-slot PSUM rotation:

  * ACT path (local cols 0..ACOLS): row-major sim chunks [128 rows, 1024].
    ScalarE exp with bias=-SHIFT and fused accum_out row-sums (esum).
    The diagonal (always in local cols [0,1024) thanks to the per-core xt2
    rotation) is killed pre-exp by a diag(-1e4) accumulate-matmul.
  * DVE path (cols ACOLS..8192): TRANSPOSED sim chunks [128 cols, 1024
    rows] (lhsT = xt2 column block, rhs = xbt). VectorE computes a
    one-instruction Schraudolph fast-exp: bits16 = int16(max(sim,0) *
    128/ln2), bitcast to bf16 == e^(sim - 127*ln2) * rho, rho in
    [1, 1.086] (measured mean 1.0410 on HW, folded into KDV). The clamp is
    mandatory: the int16 convert WRAPS on negative; clamped terms
    contribute exactly +0.0 (true value < e^-88, negligible).
    The PE row-sums the bf16 tiles with masked-ones lhsT matmuls (mask
    columns 0-63 select out-rows 0-63 for row-half 0, 64-127 for half 1)
    so BOTH halves accumulate into one standard [128, 512] PSUM bank at
    tile_position (0,0) -- col-tiled M=1 outputs proved broken. A 4x
    PE-transpose at the tail converts rows-in-free to row-major [128, 8].

  PE lanes: ACT fills use lo operand copies (array rows 0-63), DVE fills
  hi copies (rows 64-127), emitted interleaved for row-group overlap.
  Each chunk's reduce matmuls are deferred one super-step so the in-order
  PE queue never waits on DVE's fast-exp.

Everything O(N*C) -- class sums, positive-pair sums, counts, row norms,
logit gather, the CE term, and the final combine -- runs on the host in
float64 alongside the input prep (rotations/onehots/casts). This removes
the AllReduce whose enqueue-barrier (~47us of cross-core skew) + ~14us
ncfw latency gated the previous design's tail.

Outputs per core: [128, 8] se_act (shift-100 domain) and [128, 8] seD
(fast-exp domain); host computes lse = ln(se + KDV*seD) + SHIFT.
"""

import math

import numpy as np

N, C = 8192, 64
NCORES = 8
RPC = N // NCORES          # rows per core (1024)
P = 128                    # partitions per row-block
NBLK = RPC // P            # 8 row blocks per core
TAU = 0.5
LMBD = 0.5
SHIFT = 100.0              # ACT-path logsumexp shift
ACOLS = 4096               # ACT row-major columns per core
DCOLS = N - ACOLS          # DVE transposed columns per core
AW = 1024                  # chunk width
NACH = ACOLS // AW         # ACT chunks per block (4)
NKB = DCOLS // P           # DVE column blocks (32)
AEXP = 128.0 / math.log(2.0)          # fast-exp scale (184.6646)
RHO = 1.0410                          # measured mean Schraudolph ratio (HW)
KDV = math.exp(127.0 * math.log(2.0) - SHIFT) / RHO

_CACHE = {}


def _build():
    from contextlib import ExitStack

    import concourse.bass as bass
    import concourse.tile as tile
    from concourse import bacc, mybir

    f32 = mybir.dt.float32
    i16 = mybir.dt.int16
    bf16 = mybir.dt.bfloat16
    AF = mybir.ActivationFunctionType
    ALU = mybir.AluOpType
    AX = mybir.AxisListType

    nc = bacc.Bacc(
        "TRN2",
        target_bir_lowering=False,
        debug=False,
        num_devices=NCORES,
    )

    xt2a_d = nc.dram_tensor("xt2a", [C, ACOLS], bf16, kind="ExternalInput")
    xt2b_d = nc.dram_tensor("xt2b", [C, DCOLS], bf16, kind="ExternalInput")
    xbt_d = nc.dram_tensor("xbt", [C, RPC], bf16, kind="ExternalInput")
    eye_d = nc.dram_tensor("eyeneg", [P, P], bf16, kind="ExternalInput")
    idn_d = nc.dram_tensor("ident", [P, P], bf16, kind="ExternalInput")
    out_d = nc.dram_tensor("out", [P, 16], f32, kind="ExternalOutput")

    def emit(tc, ctx):
        const = ctx.enter_context(tc.tile_pool(name="const", bufs=1))
        strm = ctx.enter_context(tc.tile_pool(name="strm", bufs=3, space="PSUM"))
        accp = ctx.enter_context(tc.tile_pool(name="accp", bufs=1, space="PSUM"))
        auxp = ctx.enter_context(tc.tile_pool(name="auxp", bufs=1, space="PSUM"))
        scrp = ctx.enter_context(tc.tile_pool(name="scrp", bufs=2))
        ep = ctx.enter_context(tc.tile_pool(name="ep", bufs=3))
        stats = ctx.enter_context(tc.tile_pool(name="stats", bufs=1))

        # ---- input DMAs on two queues; first-chunk operands lead ----
        xbt_sb = const.tile([P, RPC], bf16)
        nc.sync.dma_start(xbt_sb[0:C, :], xbt_d.ap())
        nc.sync.dma_start(xbt_sb[C:P, :], xbt_d.ap())
        xt2b_sb = const.tile([P, DCOLS], bf16)
        nc.sync.dma_start(xt2b_sb[C:P, 0:AW], xt2b_d.ap()[:, 0:AW])
        xt2a_sb = const.tile([P, ACOLS], bf16)
        nc.sync.dma_start(xt2a_sb[0:C, 0:AW], xt2a_d.ap()[:, 0:AW])
        eye_sb = const.tile([P, P], bf16)
        nc.sync.dma_start(eye_sb[:], eye_d.ap())
        idn_sb = const.tile([P, P], bf16)
        nc.sync.dma_start(idn_sb[:], idn_d.ap())
        nc.sync.dma_start(xt2a_sb[0:C, AW:ACOLS], xt2a_d.ap()[:, AW:ACOLS])
        nc.sync.dma_start(xt2b_sb[C:P, AW:DCOLS], xt2b_d.ap()[:, AW:DCOLS])

        # ---- persistent small tiles ----
        nshift = stats.tile([P, 1], f32)
        nc.vector.memset(nshift[:], -SHIFT)
        # masked-ones lhsT: mh0 -> out rows 0-63 (row-half 0), mh1 -> 64-127
        masks = stats.tile([P, 2 * P], bf16)
        nc.vector.memset(masks[:, 0:C], 1.0)
        nc.vector.memset(masks[:, C:P], 0.0)
        nc.vector.memset(masks[:, P : P + C], 0.0)
        nc.vector.memset(masks[:, P + C : 2 * P], 1.0)
        acc_sb = stats.tile([P, 512], bf16)
        res = stats.tile([P, 16], f32)

        # rowsum accumulator: rows 0-63 = row-half 0 (redundant copies),
        # rows 64-127 = half 1; free = row-within-half
        acc = accp.tile([P, 512], f32, tag="acc")
        # ACT accumulator drains to PSUM (faster ScE port than SBUF); the
        # aux bank is otherwise idle until the tail
        esum = auxp.tile([P, 512], f32, tag="aux")

        # ---- main interleaved exp stream ----
        pending_acc = []

        def emit_acc(kb, eb):
            for h in range(2):
                nc.tensor.matmul(
                    acc[:, 0:512],
                    lhsT=masks[:, h * P : (h + 1) * P],
                    rhs=eb[:, h * 512 : (h + 1) * 512],
                    start=(kb == 0 and h == 0),
                    stop=(kb == NKB - 1 and h == 1),
                    skip_group_check=True,
                )

        def super_step(b, w, kb):
            ps_a = strm.tile([P, AW], f32, tag="s")
            ps_d = strm.tile([P, AW], f32, tag="s")
            for h in range(2):
                nc.tensor.matmul(
                    ps_d[:, h * 512 : (h + 1) * 512],
                    lhsT=xt2b_sb[C:P, kb * P : (kb + 1) * P],
                    rhs=xbt_sb[C:P, h * 512 : (h + 1) * 512],
                    start=True,
                    stop=True,
                )
                nc.tensor.matmul(
                    ps_a[:, h * 512 : (h + 1) * 512],
                    lhsT=xbt_sb[0:C, b * P : (b + 1) * P],
                    rhs=xt2a_sb[0:C, w * AW + h * 512 : w * AW + (h + 1) * 512],
                    start=True,
                    stop=True,
                )
            if w == 0:
                # kill self-similarity (local col b*128+p) pre-exp
                nc.tensor.matmul(
                    ps_a[:, b * P : (b + 1) * P],
                    lhsT=idn_sb[:],
                    rhs=eye_sb[:],
                    start=False,
                    stop=True,
                    skip_group_check=True,
                )
            if pending_acc:
                emit_acc(*pending_acc.pop())
            scr = scrp.tile([P, AW], bf16, tag="scr")
            idx = b * NACH + w
            nc.scalar.activation(
                scr[:], ps_a[:], AF.Exp, bias=nshift[:],
                accum_out=esum[:, idx : idx + 1],
            )
            et = ep.tile([P, AW], i16, tag="E")
            nc.vector.tensor_scalar(
                out=et[:], in0=ps_d[:], scalar1=0.0, scalar2=AEXP,
                op0=ALU.max, op1=ALU.mult,
            )
            pending_acc.append((kb, et[:].bitcast(bf16)))

        a_list = [(b, w) for b in range(NBLK) for w in range(NACH)]
        for step in range(NKB):
            b, w = a_list[step]
            super_step(b, w, step)
        while pending_acc:
            emit_acc(*pending_acc.pop())

        # ---- tail: per-block rowsums out ----
        nc.vector.reduce_sum(
            res[:, 0:NBLK],
            esum[:, 0 : NBLK * NACH].rearrange("p (b w) -> p b w", w=NACH),
            axis=AX.X,
        )
        nc.vector.tensor_copy(acc_sb[:], acc[:])
        tps = accp.tile([P, 512], bf16, tag="acc")
        for w in range(4):
            nc.tensor.transpose(
                tps[:, w * P : (w + 1) * P],
                acc_sb[:, w * P : (w + 1) * P],
                idn_sb[:],
            )
        # seD[p, b] with b = h*4 + w sits at tps[p, w*128 + h*64]
        tq = tps[:].rearrange("p (w q r) -> p w q r", w=4, q=2)
        seD_v = res[:, NBLK : 2 * NBLK].rearrange("p (h w o) -> p w h o", w=4, o=1)
        nc.vector.tensor_copy(seD_v, tq[:, :, 0:2, 0:1])
        nc.sync.dma_start(out_d.ap(), res[:])

    with tile.TileContext(nc) as tc, ExitStack() as ctx:
        emit(tc, ctx)

    nc.compile()
    return nc


def _get_nc(**kw):
    key = repr(sorted(kw.items()))
    if key not in _CACHE:
        _CACHE[key] = _build(**kw)
    return _CACHE[key]


def _make_in_maps(X, y):
    import ml_dtypes

    bf = ml_dtypes.bfloat16
    X = np.ascontiguousarray(np.asarray(X, dtype=np.float32))
    assert X.shape == (N, C)

    xt2 = np.ascontiguousarray((X.T / np.float32(TAU)).astype(bf))
    eyeneg = (np.eye(P) * -1e4).astype(bf)
    ident = np.eye(P).astype(bf)

    in_maps = []
    for r in range(NCORES):
        rows = slice(r * RPC, (r + 1) * RPC)
        xb = X[rows]
        xt2r = np.roll(xt2, -r * RPC, axis=1)
        in_maps.append(
            {
                "xt2a": np.ascontiguousarray(xt2r[:, :ACOLS]),
                "xt2b": np.ascontiguousarray(xt2r[:, ACOLS:]),
                "xbt": np.ascontiguousarray(xb.T.astype(bf)),
                "eyeneg": eyeneg,
                "ident": ident,
            }
        )
    return in_maps


def run(input, target, trace=False, **build_kw):
    """Run the device kernel; returns (loss_scalar, BassKernelResults)."""
    from concourse.bass_utils import run_bass_kernel_spmd

    nc = _get_nc(**build_kw)
    X = np.ascontiguousarray(np.asarray(input, dtype=np.float32))
    y = np.asarray(target).astype(np.int64).ravel()
    in_maps = _make_in_maps(X, y)
    res = run_bass_kernel_spmd(
        nc, in_maps, core_ids=list(range(NCORES)), trace=trace
    )

    # device gave per-row exp sums; the O(N*C) remainder runs here in f64
    se = np.empty(N)
    for r, core_out in enumerate(res.results):
        o = core_out["out"].astype(np.float64)  # [128, 16]
        se_act = o[:, 0:NBLK]    # [128, 8], row b*128+p -> [p, b]
        se_dve = o[:, NBLK:16]
        tot = se_act + KDV * se_dve              # shift-SHIFT domain
        se[r * RPC:(r + 1) * RPC] = np.maximum(tot.T.ravel(), 1e-300)

    Xd = X.astype(np.float64)
    lse = np.log(se) + SHIFT                     # [N]
    S = np.zeros((C, C))
    np.add.at(S, y, Xd)
    counts = np.bincount(y, minlength=C).astype(np.float64)
    n_pos = counts[y] - 1.0
    possum = ((Xd * S[y]).sum(axis=1) - (Xd * Xd).sum(axis=1)) / TAU
    per_i = np.where(n_pos > 0, lse - possum / np.maximum(n_pos, 1.0), 0.0)
    sc = per_i.sum()

    m = Xd.max(axis=1)
    ce_lse = np.log(np.exp(Xd - m[:, None]).sum(axis=1)) + m
    ce = (ce_lse - Xd[np.arange(N), y]).mean()

    loss = (1.0 - LMBD) * ce + LMBD * sc
    return np.array(loss, dtype=np.float32), res


def kernel(input, target):
    loss, _ = run(input, target, trace=False)
    return loss
